# revision 22
# baseline (speedup 1.0000x reference)
"""Trainium2 Bass kernel for the Autoformer autocorrelation block.

Contract: kernel(**inputs) takes FULL inputs (B=8 batches), returns FULL output
[8, 3072, 1024] f32. Internally: data-parallel over batch across 8 NeuronCores.

Weight folding (host side, fp64): the correlation only needs circular
diag-sums of Q K^T = Xq (Wq Wk^T) Xk^T + bias terms that are constant in the
delay, and top-k + softmax are invariant to constant shifts — so Wqk = Wq Wk^T
replaces the Q AND K projections with a single one.  The delay aggregation is
a convex combination of time shifts and commutes with the channel projection,
so Wvo = Wv Wo (bvo = bv Wo + bo) fuses the V and O projections.  PE work
drops from 5 to 2 projections + the Gram (22.5 -> 16.1 GMAC per core).

Per-core algorithm (one batch; t = time in [0,3072), d = channel in [0,1024)):
  1. XBAR DMA-transpose Xq (thirds, pipelined) and project with fp16 Wqk on
     the PE -> Q'^T in [d, t] layout; Xk^T and Xv^T are straight XBAR DMA
     transposes (no matmuls).  Issue order q -> k -> v matches consumer
     order since all transposes serialize on the shared XBAR path.
  2. diag-sums via Gram tiles on PE with block-diagonal ring accumulation
     (ring[jj] = sum of [128,128] blocks with (b-a)%24 == jj), then a
     strided-DMA "skew" through DRAM turns diagonals into columns and a
     PE ones-matmul reduce yields all 3072 diagonal sums at once (the dead
     ring tile's first row is reused as the colsum buffer).
  3. top-8 values+indices via DVE max/max_index; softmax on-device; delay
     values recovered with register ALU (DVE + ACT register sets).
  4. P^T = Wvo^T Xv^T + bvo (fp16 matmuls), written doubled along t from
     PSUM (ACT), per m-tile.
  5. out^T[d, t] = sum_i w_i * P^T[d, t + d_i] via runtime-register dynamic
     slices: 4x-mode tensor_scalar scales (DVE, taps 1-2 on ACT) + 2x
     tensor_tensor adds, pipelined per m-tile against the P-projection.
     Host transposes back and upcasts fp16 -> f32.

Measured (loop-differenced, 8 cores): 702us baseline -> 572us.  Rejected by
measurement: Pool-engine merge adds in the combine (cross-engine semaphore
serialization, +90us), pre-emitting P-proj m-tiles before the colsum, and
DMA-based P^T doubling (both slightly negative).  fp8 Gram is numerically
unsafe: the rank-8/9 correlation gap is ~0.0025 sigma_c and fp8 input
quantization noise (~5% sigma_c) would flip top-k taps (output error ~0.5).

Timing support: build_nc(kiter=K) emits the body K times separated by
all-engine barriers, so test.py can measure the marginal per-iteration
hardware execution time ((t_K - t_1) / (K - 1)) with dispatch overhead
cancelled.
"""
import os
import sys

if "/opt/trn_rl_repo" not in sys.path:
    sys.path.insert(0, "/opt/trn_rl_repo")

import numpy as np

import concourse.bacc as bacc
import concourse.mybir as mybir
import concourse.tile as tile
from concourse.bass import ds
from concourse.bass_types import AP
from concourse.masks import make_identity

B, L, D = 8, 3072, 1024
NT = L // 128          # 24 t-blocks
NC = L // 512          # 6 t-chunks
KT = D // 128          # 8 contraction tiles
MT = D // 128          # 8 output-channel tiles
TOPK = 8
N_CORES = 8
WG = 3200              # ring width incl prepended block (25*128)
WS = WG + 127          # skew row width

F32 = mybir.dt.float32
F16 = mybir.dt.float16
U32 = mybir.dt.uint32
AF = mybir.ActivationFunctionType
ALU = mybir.AluOpType

# row offsets of q/k/v in xpack, and of Wqk/Wvo in wpack.
# Weight folding: diag-sums of Q K^T equal diag-sums of Xq (Wq Wk^T) Xk^T up
# to a delay-independent constant (bias terms), which softmax and top-k both
# ignore; and the output is sum_i w_i shift_{d_i}(Xv (Wv Wo)) + (bv Wo + bo)
# because the delay aggregation commutes with the channel projection and
# sum_i w_i = 1.  So only TWO projections remain: Q' = Xq Wqk and
# P = Xv Wvo + bvo.
XOFF = {"q": 0, "k": 1, "v": 2}
WOFF = {"qk": 0, "vo": 1}


def build_nc(kiter=1):
    nc = bacc.Bacc("TRN2", target_bir_lowering=False, debug=False,
                   num_devices=N_CORES)

    aps = {
        "xpack": nc.dram_tensor("xpack", [3 * L, D], F16,
                                kind="ExternalInput").ap(),
        "wpack": nc.dram_tensor("wpack", [2 * D, D], F16,
                                kind="ExternalInput").ap(),
        "bpack": nc.dram_tensor("bpack", [1, D], F32,
                                kind="ExternalInput").ap(),
    }
    out = nc.dram_tensor("out", [D, L], F16, kind="ExternalOutput").ap()
    skew = nc.dram_tensor("skew", [128 * WS + 256], F32)
    with tile.TileContext(nc) as tc:
        for it in range(kiter):
            _kernel_body(tc, nc, aps, out, skew, itag=str(it))
            if it < kiter - 1:
                tc.strict_bb_all_engine_barrier()
    nc.compile()
    return nc


def _load_weights16(nc, pool, w_dram, tag):
    """W [din, dout] fp16 -> SBUF fp16 [128, KT*D]; w16[p, kt*D+n] = W[kt*128+p, n]."""
    w16 = pool.tile([128, KT * D], F16, tag="w16", name=f"w16_{tag}")
    nc.sync.dma_start(w16.rearrange("p (a n) -> p a n", a=KT),
                      w_dram.rearrange("(a p) n -> p a n", p=128))
    return w16


def _transpose_chunk_dma(nc, x_dram, x_base, c, xtp):
    """XBAR DMA-transpose fp16 x rows [512c, 512(c+1)) straight from DRAM into
    xtp [128, KT*512] with xtp[p, kt*512 + j] = x[x_base + 512c + j, kt*128+p].

    KTRSPLIT=1 alternates issues between the SP and ACT HWDGE queues —
    measured INCORRECT output (ACT-issued transpose XBAR DMAs corrupt the
    result), so it stays off."""
    split = int(os.environ.get("KTRSPLIT", "0"))
    for kt in range(KT):
        eng = nc.scalar if (split and kt % 2) else nc.sync
        eng.dma_start_transpose(
            xtp[:, 512 * kt:512 * (kt + 1)],
            x_dram[x_base + 512 * c: x_base + 512 * (c + 1),
                   128 * kt:128 * (kt + 1)])


def _transpose_chunk(nc, ident, x_dram, x_base, c, xin_pool, tpsum_pool, xtp,
                     itag):
    """PE-transpose fp16 x rows [512c, 512(c+1)) into xtp [128, KT*512] with
    xtp[p, kt*512 + al*128 + j] = x[x_base + 512c + al*128 + j, kt*128 + p]."""
    for al in range(4):
        a = 4 * c + al
        x16 = xin_pool.tile([128, D], F16, tag="x16",
                            name=f"x16_{c}_{al}_{itag}")
        nc.sync.dma_start(x16, x_dram[x_base + 128 * a:x_base + 128 * (a + 1), :])
        for half in range(2):
            pt = tpsum_pool.tile([128, 512], F16, tag="tp",
                                 name=f"pt_{c}_{al}_{half}_{itag}")
            for k2 in range(4):
                dt = 4 * half + k2
                nc.tensor.transpose(
                    pt[:, 128 * k2:128 * (k2 + 1)],
                    x16[:, 128 * dt:128 * (dt + 1)],
                    ident,
                )
            dst = xtp.rearrange("p (k f) -> p k f", f=512)[
                :, 4 * half:4 * half + 4, 128 * al:128 * (al + 1)]
            src = pt.rearrange("p (k f) -> p k f", f=128)
            nc.vector.tensor_copy(dst, src)


def _load_bias(nc, pool, b_dram, tag):
    """bias [1, D] f32 -> SBUF [128, MT]; b_sb[p, m] = bias[m*128+p]."""
    b_sb = pool.tile([128, MT], F32, tag=tag, name=f"b_{tag}")
    nc.sync.dma_start(b_sb, b_dram.rearrange("o (m p) -> (o p) m", p=128))
    return b_sb


def _kernel_body(tc, nc, aps, out, skew, itag="0"):
    import contextlib
    PHASES = int(os.environ.get("KPHASES", "9"))
    est = contextlib.ExitStack()

    xpack, wpack, bpack = aps["xpack"], aps["wpack"], aps["bpack"]

    bias_pool = est.enter_context(tc.tile_pool(name=f"bias{itag}", bufs=1))
    small_pool = est.enter_context(tc.tile_pool(name=f"small{itag}", bufs=1))
    kv_pool = est.enter_context(tc.tile_pool(name=f"kv{itag}", bufs=1))
    ring_pool = est.enter_context(tc.tile_pool(name=f"ring{itag}", bufs=1))
    est_kt = contextlib.ExitStack()
    kt_pool = est_kt.enter_context(tc.tile_pool(name=f"ktp{itag}", bufs=1))
    qt_pool = est_kt.enter_context(tc.tile_pool(name=f"qtp{itag}", bufs=1))

    b_sb = {"o": _load_bias(nc, bias_pool, bpack[0:1, :], f"bo_{itag}")}

    kt_sb = kt_pool.tile([128, MT * L], F16, tag="kt",
                         name=f"kt_sb_{itag}")    # Xk^T, m-major
    qt_sb = qt_pool.tile([128, MT * L], F16, tag="qt",
                         name=f"qt_sb_{itag}")    # Q'^T = (Xq Wqk)^T, m-major
    vt_sb = kv_pool.tile([128, MT * L], F16, tag="vt",
                         name=f"vt_sb_{itag}")    # Xv^T, m-major

    # ------- Phase 1: Xq^T transpose + Q' projection; Xk^T/Xv^T transpose ---
    TH = L // 3
    with tc.tile_pool(name=f"wpool{itag}", bufs=1) as wpool, \
         tc.tile_pool(name=f"xtp{itag}", bufs=2) as xtp_pool, \
         tc.tile_pool(name=f"ppsum{itag}", bufs=4, space="PSUM") as ppsum_pool:
        w16 = _load_weights16(nc, wpool,
                              wpack[WOFF["qk"] * D:(WOFF["qk"] + 1) * D, :],
                              f"wqk_{itag}")
        # Xq first (its transposes gate the only phase-1 PE work), then Xk
        # (gates the Gram), then Xv (only needed by the P-projection, which
        # runs after the Gram).  All XBAR transposes serialize on the shared
        # DMA/XBAR path (~43us per signal), so issue order = consumer order.
        x_base = XOFF["q"] * L
        for h in range(3):
            xtp = xtp_pool.tile([128, KT * TH], F16, tag="xtp3",
                                name=f"xtp3_q_{h}_{itag}")
            for kt in range(KT):
                nc.sync.dma_start_transpose(
                    xtp[:, TH * kt:TH * (kt + 1)],
                    xpack[x_base + TH * h: x_base + TH * (h + 1),
                          128 * kt:128 * (kt + 1)])
            for cc in range(2):
                c = 2 * h + cc
                for m in range(MT):
                    pp = ppsum_pool.tile([128, 512], F32, tag="pp",
                                         name=f"pp_q_{c}_{m}_{itag}")
                    for kt in range(KT):
                        nc.tensor.matmul(
                            pp,
                            w16[:, kt * D + 128 * m:
                                kt * D + 128 * (m + 1)],
                            xtp[:, TH * kt + 512 * cc:
                                TH * kt + 512 * (cc + 1)],
                            start=(kt == 0), stop=(kt == KT - 1),
                        )
                    nc.scalar.activation(
                        qt_sb[:, m * L + 512 * c: m * L + 512 * (c + 1)],
                        pp, AF.Identity, bias=0.0, scale=1.0)
        # Xk^T / Xv^T: straight XBAR DMA transposes into kt_sb / vt_sb
        # (no projection matmuls — the Gram consumes raw Xk^T and the
        # P-projection consumes raw Xv^T with the folded Wvo).
        for which, dst in (("k", kt_sb), ("v", vt_sb)):
            x_base = XOFF[which] * L
            for h in range(3):
                for kt in range(KT):
                    nc.sync.dma_start_transpose(
                        dst[:, kt * L + TH * h: kt * L + TH * (h + 1)],
                        xpack[x_base + TH * h: x_base + TH * (h + 1),
                              128 * kt:128 * (kt + 1)])

    if PHASES < 2:
        est_kt.close(); est.close(); return

    # ---------------- Phase 2: Gram + block-diagonal ring ----------------
    ring = ring_pool.tile([128, WG], F32, tag="ring", name=f"ring_{itag}")
    if int(os.environ.get("KPSRING", "1")):
        # Accumulate the ring DIRECTLY in PSUM across a-tiles: ring column
        # rc (slot jj = rc//128 - 1, lane u = rc%128) needs
        #   sum_a sum_d qt[d, 128a+p] * kt[d, (128(a-1) + rc) % L],
        # i.e. for each (chunk, a, kt) one moving slice of Xk^T that is
        # contiguous except at the mod-L wrap (<= 2 matmul pieces).  This
        # removes all 144 DVE ring drains and their PSUM-reuse stalls; every
        # column's first/last contribution is at a=0 / a=NT-1, so start/stop
        # flags are uniform per a.
        with tc.tile_pool(name=f"gpsum{itag}", bufs=2,
                          space="PSUM") as gpsum_pool:
            for h in range((WG + 511) // 512):
                rc0 = 512 * h
                w = min(512, WG - rc0)
                rg = gpsum_pool.tile([128, w], F32, tag="rg",
                                     name=f"rg{h}_{itag}")
                # rotate the a-loop so the FIRST step is single-piece: the
                # start bit zeroes the whole target region, so a two-piece
                # first step would wipe its own first piece.
                a_start = next(a for a in range(NT)
                               if (128 * (a - 1) + rc0) % L + w <= L)
                for ai in range(NT):
                    a = (a_start + ai) % NT
                    for kt in range(KT):
                        t0 = (128 * (a - 1) + rc0) % L
                        if t0 + w <= L:
                            pieces = ((0, t0, w),)
                        else:
                            w1 = L - t0
                            pieces = ((0, t0, w1), (w1, 0, w - w1))
                        for po, ts, pwid in pieces:
                            nc.tensor.matmul(
                                rg[:, po:po + pwid],
                                qt_sb[:, kt * L + 128 * a:
                                      kt * L + 128 * (a + 1)],
                                kt_sb[:, kt * L + ts: kt * L + ts + pwid],
                                start=(ai == 0 and kt == 0),
                                stop=(ai == NT - 1 and kt == KT - 1),
                                skip_group_check=True,
                            )
                nc.vector.tensor_copy(ring[:, rc0:rc0 + w], rg)
        est_kt.close()  # K^T / Q'^T no longer needed
        if PHASES < 3:
            est.close(); return
        return _phase345(tc, nc, est, out, skew, ring, vt_sb, b_sb,
                         small_pool, wpack, itag)

    nc.vector.memset(ring, 0.0)
    with tc.tile_pool(name=f"gpsum{itag}", bufs=1, space="PSUM") as gpsum_pool:
        for a in range(NT):
            gps = [gpsum_pool.tile([128, 512], F32, tag=f"gp{c}",
                                   name=f"gp{a}_{c}_{itag}")
                   for c in range(NC)]
            if int(os.environ.get("KCMAJ", "1")):
                # c-major: each psum tile finishes early so its ring add
                # (DVE) overlaps the next tile's matmuls instead of
                # stalling the a+1 accumulation group on psum reuse.
                for c in range(NC):
                    for kt in range(KT):
                        nc.tensor.matmul(
                            gps[c],
                            qt_sb[:, kt * L + 128 * a:
                                  kt * L + 128 * (a + 1)],
                            kt_sb[:, kt * L + 512 * c:
                                  kt * L + 512 * (c + 1)],
                            start=(kt == 0), stop=(kt == KT - 1),
                        )
            else:
                for kt in range(KT):
                    for c in range(NC):
                        nc.tensor.matmul(
                            gps[c],
                            qt_sb[:, kt * L + 128 * a:
                                  kt * L + 128 * (a + 1)],
                            kt_sb[:, kt * L + 512 * c:
                                  kt * L + 512 * (c + 1)],
                            start=(kt == 0), stop=(kt == KT - 1),
                        )
            if int(os.environ.get("KNORING", "0")):
                # timing-only diagnostic: drop the ring drains entirely to
                # measure the pure Gram matmul stream (output is garbage)
                continue
            RD = int(os.environ.get("KRDRAIN", "0"))

            def radd(dst, in1, gp_slice, which):
                # KRDRAIN: split every drain column-wise between DVE and
                # Pool.  The two engines touch disjoint column ranges, so
                # each engine's RAW chain on `ring` stays engine-local (no
                # cross-engine semaphores on the accumulation path).
                if not RD:
                    nc.vector.tensor_add(dst, in1, gp_slice)
                    return
                n = dst.shape[-1]
                h = (n // 2) // 128 * 128 or n
                nc.vector.tensor_add(dst[:, :h], in1[:, :h], gp_slice[:, :h])
                if h < n:
                    nc.gpsimd.tensor_add(dst[:, h:], in1[:, h:],
                                         gp_slice[:, h:])

            for c in range(NC):
                gp = gps[c]
                jj0 = (4 * c - a) % NT
                off = 128 * (jj0 + 1)
                if jj0 <= NT - 4:
                    radd(ring[:, off:off + 512], ring[:, off:off + 512], gp,
                         c)
                else:
                    w1 = 128 * (NT - jj0)
                    radd(ring[:, off:off + w1], ring[:, off:off + w1],
                         gp[:, :w1], c)
                    radd(ring[:, 128:128 + 512 - w1],
                         ring[:, 128:128 + 512 - w1], gp[:, w1:], c)
    # ring block jj lives at offset 128*(jj+1); prepend a copy of block 23
    nc.vector.tensor_copy(ring[:, 0:128], ring[:, 128 * NT:128 * (NT + 1)])
    est_kt.close()  # K^T / Q'^T no longer needed
    if PHASES < 3:
        est.close(); return
    return _phase345(tc, nc, est, out, skew, ring, vt_sb, b_sb, small_pool,
                     wpack, itag)


def _phase345(tc, nc, est, out, skew, ring, vt_sb, b_sb, small_pool, wpack,
              itag):
    # ------ Phases 3-5: P-projection overlapped with skew -> top-8 ---------
    # The skew DMA round trip + colsum + max/softmax/register chain is a
    # serial ~25us tail that would idle the PE: emit the first KPRE m-tiles
    # of the P-projection BEFORE the colsum so the PE stays busy through it.
    PHASES = int(os.environ.get("KPHASES", "9"))
    PRE = max(0, min(int(os.environ.get("KPRE", "0")), MT))
    KDBL = int(os.environ.get("KDBLACT", "1"))
    with tc.tile_pool(name=f"wos{itag}", bufs=1) as wos_pool, \
         tc.tile_pool(name=f"p2tp{itag}", bufs=3) as p2t_pool, \
         tc.tile_pool(name=f"ppsum4{itag}", bufs=3, space="PSUM") as ppsum_pool, \
         tc.tile_pool(name=f"accp{itag}", bufs=2) as acc_pool, \
         tc.tile_pool(name=f"skp{itag}", bufs=1) as sk_pool, \
         tc.tile_pool(name=f"cspsum{itag}", bufs=1, space="PSUM") as cs_pool:
        wo16 = _load_weights16(nc, wos_pool,
                               wpack[WOFF["vo"] * D:(WOFF["vo"] + 1) * D, :],
                               f"wo_{itag}")

        def pproj(m):
            """P^T m-tile: matmuls + bias, doubled along t for the runtime
            circular slice (double via DMA on the otherwise idle SP queue
            unless KDBLACT=1 re-enables the ACT double-write)."""
            p2t = p2t_pool.tile([128, 2 * L], F16, tag="p2t",
                                name=f"p2t_{m}_{itag}")
            for c in range(NC):
                pp = ppsum_pool.tile([128, 512], F32, tag="pp",
                                     name=f"pp4_{c}_{m}_{itag}")
                for kt in range(KT):
                    nc.tensor.matmul(
                        pp,
                        wo16[:, kt * D + 128 * m: kt * D + 128 * (m + 1)],
                        vt_sb[:, kt * L + 512 * c: kt * L + 512 * (c + 1)],
                        start=(kt == 0), stop=(kt == KT - 1),
                    )
                nc.scalar.activation(
                    p2t[:, 512 * c: 512 * (c + 1)],
                    pp, AF.Identity, bias=b_sb["o"][:, m:m + 1], scale=1.0)
                if KDBL:
                    nc.scalar.activation(
                        p2t[:, L + 512 * c: L + 512 * (c + 1)],
                        pp, AF.Identity, bias=b_sb["o"][:, m:m + 1],
                        scale=1.0)
            if not KDBL:
                nc.sync.dma_start(p2t[:, L:2 * L], p2t[:, 0:L])
            return p2t

        p2ts = {}
        for m in range(PRE):
            p2ts[m] = pproj(m)

        # ---- skew -> colsum -> top-8 (DMA/PE-colsum/DVE under P-proj) ----
        sk_sb = sk_pool.tile([128, L], F32, tag="sk", name=f"sk_{itag}")
        skew_rd = AP(tensor=skew, offset=128, ap=[[WS, 128], [1, L]])
        skew_wr = AP(tensor=skew, offset=127, ap=[[WS - 1, 128], [1, WG]])
        nc.sync.dma_start(skew_wr, ring[:, 0:WG])    # skewed write
        nc.sync.dma_start(sk_sb, skew_rd)            # read back
        ones = sk_pool.tile([128, 1], F32, tag="ones", name=f"ones_{itag}")
        nc.vector.memset(ones, 1.0)
        # ring is dead once the skew write has read it — reuse its first row
        # as the colsum buffer (Tile orders the WAR hazard on the slice).
        colsum = ring[0:1, 0:L]
        for half in range(2):
            cs_psum = cs_pool.tile([1, L // 2], F32, tag="cs",
                                   name=f"cs_{half}_{itag}")
            for ch in range(NC // 2):
                nc.tensor.matmul(
                    cs_psum[:, 512 * ch:512 * (ch + 1)],
                    ones,
                    sk_sb[:, half * (L // 2) + 512 * ch:
                          half * (L // 2) + 512 * (ch + 1)],
                    start=True, stop=True,
                )
            nc.vector.tensor_copy(colsum[:, half * (L // 2):
                                         (half + 1) * (L // 2)], cs_psum)
        max8 = small_pool.tile([1, TOPK], F32, tag="max8", name=f"max8_{itag}")
        idx8 = small_pool.tile([1, TOPK], U32, tag="idx8", name=f"idx8_{itag}")
        sl = colsum[0:1, 0:L]
        nc.vector.max(out=max8, in_=sl)
        nc.vector.max_index(idx8, max8, sl)
        if PHASES < 4:
            est.close(); return

        # softmax(max8 / D)
        wts = small_pool.tile([1, TOPK], F32, tag="wts", name=f"wts_{itag}")
        negmax = small_pool.tile([1, 1], F32, tag="negmax",
                                 name=f"negmax_{itag}")
        inv = small_pool.tile([1, 1], F32, tag="inv", name=f"inv_{itag}")
        nc.vector.tensor_scalar_mul(negmax, max8[0:1, 0:1], -1.0 / D)
        nc.scalar.activation(wts, max8, AF.Exp, bias=negmax[0:1, 0:1],
                             scale=1.0 / D)
        nc.vector.reduce_sum(inv, wts, axis=mybir.AxisListType.X)
        nc.vector.reciprocal(inv, inv)
        nc.vector.tensor_scalar(wts, wts, inv[0:1, 0:1], None, op0=ALU.mult)
        w_bc = small_pool.tile([128, TOPK], F32, tag="wbc", name=f"wbc_{itag}")
        nc.gpsimd.partition_broadcast(w_bc, wts)

        # delay regs: m = idx; jd = m>>7; u = 127 - m%128; delta = (24-jd)%24;
        # d = 128*delta + u.  One register set per engine, only for the taps
        # that engine actually combines.
        engines = {"v": mybir.EngineType.DVE,
                   "a": mybir.EngineType.Activation}
        tap_sets = {"v": (0, 3, 4, 5, 6, 7), "a": (1, 2)}
        delay_sv = {}
        for key, etype in engines.items():
            eng = nc.engines[etype]
            svs = {}
            for i in tap_sets[key]:
                regs = nc.alloc_registers(f"dly{key}{i}i{itag}", (etype,))
                nc.regs_load(regs, idx8[0:1, i:i + 1])
                r0 = regs.handles[0]
                t1 = eng.alloc_register(f"t1{key}_{i}_{itag}")
                t2 = eng.alloc_register(f"t2{key}_{i}_{itag}")
                eng.reg_alu(t1, r0, 128, ALU.divide)      # jd
                eng.reg_alu(t2, t1, 128, ALU.mult)
                eng.reg_alu(r0, r0, t2, ALU.subtract)     # m % 128
                eng.reg_alu(r0, 127, r0, ALU.subtract)    # u
                eng.reg_alu(t1, NT, t1, ALU.subtract)     # 24 - jd
                eng.reg_alu(t1, t1, NT, ALU.mod)          # delta
                eng.reg_alu(t1, t1, 128, ALU.mult)
                eng.reg_alu(t1, t1, r0, ALU.add)          # d
                svs[i] = nc.snap(t1, min_val=0, max_val=L - 1)
            delay_sv[key] = svs

        # ---- remaining P-proj m-tiles + per-m combine (DVE/ACT/Pool) ----
        def combine(m, p2t):
            svs = delay_sv["v"]
            asvs = delay_sv["a"]
            acc = acc_pool.tile([128, L], F16, tag="acc", name=f"acc_{m}_{itag}")
            t_a = acc_pool.tile([128, L], F16, tag="t_a", name=f"ta_{m}_{itag}")
            t_b = acc_pool.tile([128, L], F16, tag="t_b", name=f"tb_{m}_{itag}")
            t4 = acc_pool.tile([128, L], F16, tag="t4", name=f"t4_{m}_{itag}")
            pw = p2t[:, 0:2 * L]
            # ACT: taps 1, 2 (activation scale)
            nc.scalar.activation(t_a, pw[:, ds(asvs[1], L)], AF.Identity,
                                 bias=0.0, scale=w_bc[:, 1:2])
            nc.scalar.activation(t_b, pw[:, ds(asvs[2], L)], AF.Identity,
                                 bias=0.0, scale=w_bc[:, 2:3])
            # DVE: all pointer-scaled taps (TensorScalarPtr is not legal on
            # Pool); taps 3,4 go to scratch tiles so the Pool engine can take
            # over 3 of the 7 merge adds (plain TensorTensor is Pool-legal).
            nc.vector.tensor_scalar(acc, pw[:, ds(svs[0], L)],
                                    w_bc[:, 0:1], None, op0=ALU.mult)
            for i in (5, 6, 7):
                nc.vector.tensor_scalar(t4, pw[:, ds(svs[i], L)],
                                        w_bc[:, i:i + 1], None, op0=ALU.mult)
                nc.vector.tensor_add(acc, acc, t4)
            if int(os.environ.get("KGPS", "0")):
                t3s = acc_pool.tile([128, L], F16, tag="t3s",
                                    name=f"t3s_{m}_{itag}")
                t4s = acc_pool.tile([128, L], F16, tag="t4s",
                                    name=f"t4s_{m}_{itag}")
                nc.vector.tensor_scalar(t3s, pw[:, ds(svs[3], L)],
                                        w_bc[:, 3:4], None, op0=ALU.mult)
                nc.vector.tensor_scalar(t4s, pw[:, ds(svs[4], L)],
                                        w_bc[:, 4:5], None, op0=ALU.mult)
                nc.gpsimd.tensor_add(t3s, t3s, t4s)
                nc.gpsimd.tensor_add(t3s, t3s, t_a)
                nc.gpsimd.tensor_add(t3s, t3s, t_b)
                nc.vector.tensor_add(acc, acc, t3s)
            else:
                for i in (3, 4):
                    nc.vector.tensor_scalar(t4, pw[:, ds(svs[i], L)],
                                            w_bc[:, i:i + 1], None,
                                            op0=ALU.mult)
                    nc.vector.tensor_add(acc, acc, t4)
                nc.vector.tensor_add(t_a, t_a, t_b)
                nc.vector.tensor_add(acc, acc, t_a)
            nc.sync.dma_start(out[128 * m:128 * (m + 1), :], acc)

        COMBM = int(os.environ.get("KCOMBM", "8"))
        for m in range(MT):
            p2t = p2ts[m] if m in p2ts else pproj(m)
            if m < COMBM:
                combine(m, p2t)

    est.close()


# ------------------------- host-side wrapper -------------------------
_CACHE = {}


def _build_runner(kiter=1, donate=True):
    """Build nc + a cached jitted SPMD callable (mirrors run_bass_via_pjrt).

    donate=False keeps the zero output buffers as ordinary (reusable) inputs:
    the kernel writes every element of `out`, so the pre-zeroed donation is
    only an XLA aliasing optimization, not a correctness requirement.  Timing
    harnesses use donate=False so staged device arrays can be reused across
    back-to-back dispatches."""
    import jax
    from jax.sharding import Mesh, PartitionSpec
    from jax.experimental.shard_map import shard_map
    from concourse import bass2jax
    import concourse.mybir as mb

    nc = build_nc(kiter=kiter)
    bass2jax.install_neuronx_cc_hook()

    partition_name = (nc.partition_id_tensor.name
                      if nc.partition_id_tensor else None)
    in_names, out_names, out_avals, zero_outs = [], [], [], []
    for alloc in nc.m.functions[0].allocations:
        if not isinstance(alloc, mb.MemoryLocationSet):
            continue
        name = alloc.memorylocations[0].name
        if alloc.kind == "ExternalInput":
            if name != partition_name:
                in_names.append(name)
        elif alloc.kind == "ExternalOutput":
            shape = tuple(alloc.tensor_shape)
            dtype = mb.dt.np(alloc.dtype)
            out_names.append(name)
            out_avals.append(jax.core.ShapedArray(shape, dtype))
            zero_outs.append(np.zeros(shape, dtype))
    n_params = len(in_names)
    all_names = list(in_names) + list(out_names)
    if partition_name is not None:
        all_names.append(partition_name)
    donate_nums = (tuple(range(n_params, n_params + len(out_names)))
                   if donate else ())

    def _body(*args):
        operands = list(args)
        if partition_name is not None:
            operands.append(bass2jax.partition_id_tensor())
        return tuple(bass2jax._bass_exec_p.bind(
            *operands,
            out_avals=tuple(out_avals),
            in_names=tuple(all_names),
            out_names=tuple(out_names),
            lowering_input_output_aliases=(),
            sim_require_finite=True,
            sim_require_nnan=True,
            nc=nc,
        ))

    devices = jax.devices()[:N_CORES]
    mesh = Mesh(np.asarray(devices), ("core",))
    in_specs = (PartitionSpec("core"),) * (n_params + len(out_names))
    out_specs = (PartitionSpec("core"),) * len(out_names)
    sharded = jax.jit(
        shard_map(_body, mesh=mesh, in_specs=in_specs, out_specs=out_specs,
                  check_rep=False),
        donate_argnums=donate_nums, keep_unused=True)
    return {
        "sharded": sharded, "in_names": in_names, "out_names": out_names,
        "out_avals": out_avals, "zero_outs": zero_outs,
    }


def _get_runner(kiter=1, donate=True):
    key = (kiter, donate)
    if key not in _CACHE:
        _CACHE[key] = _build_runner(kiter=kiter, donate=donate)
    return _CACHE[key]


def _concat_inputs(r, in_maps):
    per_core = [[np.asarray(m[name]) for name in r["in_names"]]
                for m in in_maps]
    concat_in = [np.concatenate([per_core[c][i] for c in range(N_CORES)],
                                axis=0)
                 for i in range(len(r["in_names"]))]
    concat_zeros = [np.zeros((N_CORES * z.shape[0], *z.shape[1:]), z.dtype)
                    for z in r["zero_outs"]]
    return concat_in, concat_zeros


def _run(r, concat_in, concat_zeros):
    out_arrs = r["sharded"](*concat_in, *concat_zeros)
    return [
        {name: np.asarray(out_arrs[i]).reshape(
            N_CORES, *r["out_avals"][i].shape)[c]
         for i, name in enumerate(r["out_names"])}
        for c in range(N_CORES)
    ]


def make_in_maps(queries, keys, values, Wq, bq, Wk, bk, Wv, bv, Wo, bo):
    """Pack full f32 inputs into per-core fp16 in_maps with folded weights.

    Wqk = Wq Wk^T: diag-sums of Q K^T equal diag-sums of Xq Wqk Xk^T up to a
    delay-independent constant (the bias cross-terms), which top-k ordering
    and softmax are both invariant to.  Wvo = Wv Wo (+ bvo = bv Wo + bo):
    the delay aggregation is a convex combination of time shifts and commutes
    with the channel projection, so V-proj and O-proj fuse into one matmul.
    """
    Wqk = (np.asarray(Wq, np.float64) @ np.asarray(Wk, np.float64).T)
    Wvo = (np.asarray(Wv, np.float64) @ np.asarray(Wo, np.float64))
    bvo = (np.asarray(bv, np.float64) @ np.asarray(Wo, np.float64)
           + np.asarray(bo, np.float64))
    wpack = np.concatenate([Wqk, Wvo], axis=0).astype(np.float16)
    bpack = bvo[None, :].astype(np.float32)
    queries = np.asarray(queries, np.float32)
    keys = np.asarray(keys, np.float32)
    values = np.asarray(values, np.float32)
    in_maps = []
    for b in range(B):
        xpack = np.concatenate(
            [queries[b], keys[b], values[b]], axis=0).astype(np.float16)
        in_maps.append({"xpack": xpack, "wpack": wpack, "bpack": bpack})
    return in_maps


def kernel(queries, keys, values, Wq, bq, Wk, bk, Wv, bv, Wo, bo):
    r = _get_runner(kiter=1)
    in_maps = make_in_maps(queries, keys, values, Wq, bq, Wk, bk, Wv, bv,
                           Wo, bo)
    concat_in, concat_zeros = _concat_inputs(r, in_maps)
    results = _run(r, concat_in, concat_zeros)
    outs = [results[b]["out"].T.astype(np.float32) for b in range(B)]
    return np.ascontiguousarray(np.stack(outs))


if __name__ == "__main__":
    rng = np.random.default_rng(0)
    ins = {
        "queries": rng.standard_normal((B, L, D)).astype(np.float32),
        "keys": rng.standard_normal((B, L, D)).astype(np.float32),
        "values": rng.standard_normal((B, L, D)).astype(np.float32),
        "Wq": (rng.standard_normal((D, D)) * 0.02).astype(np.float32),
        "bq": np.zeros(D, np.float32),
        "Wk": (rng.standard_normal((D, D)) * 0.02).astype(np.float32),
        "bk": np.zeros(D, np.float32),
        "Wv": (rng.standard_normal((D, D)) * 0.02).astype(np.float32),
        "bv": np.zeros(D, np.float32),
        "Wo": (rng.standard_normal((D, D)) * 0.02).astype(np.float32),
        "bo": np.zeros(D, np.float32),
    }
    o = kernel(**ins)
    print("out", o.shape, o.dtype, float(np.abs(o).max()))



# revision 23
# speedup vs baseline: 1.0405x; 1.0405x over previous
"""Trainium2 Bass kernel for the Autoformer autocorrelation block.

Contract: kernel(**inputs) takes FULL inputs (B=8 batches), returns FULL output
[8, 3072, 1024] f32. Internally: data-parallel over batch across 8 NeuronCores.

Weight folding (host side, fp64): the correlation only needs circular
diag-sums of Q K^T = Xq (Wq Wk^T) Xk^T + bias terms that are constant in the
delay, and top-k + softmax are invariant to constant shifts — so Wqk = Wq Wk^T
replaces the Q AND K projections with a single one.  The delay aggregation is
a convex combination of time shifts and commutes with the channel projection,
so Wvo = Wv Wo (bvo = bv Wo + bo) fuses the V and O projections.  PE work
drops from 5 to 2 projections + the Gram (22.5 -> 16.1 GMAC per core).

Per-core algorithm (one batch; t = time in [0,3072), d = channel in [0,1024)):
  1. XBAR DMA-transpose Xq (thirds, pipelined) and project with fp16 Wqk on
     the PE -> Q'^T in [d, t] layout; Xk^T and Xv^T are straight XBAR DMA
     transposes (no matmuls).  Issue order q -> k -> v matches consumer
     order since all transposes serialize on the shared XBAR path.
  2. diag-sums via Gram tiles on PE with block-diagonal ring accumulation
     (ring[jj] = sum of [128,128] blocks with (b-a)%24 == jj), then a
     strided-DMA "skew" through DRAM turns diagonals into columns and a
     PE ones-matmul reduce yields all 3072 diagonal sums at once (the dead
     ring tile's first row is reused as the colsum buffer).
  3. top-8 values+indices via DVE max/max_index; softmax on-device; delay
     values recovered with register ALU (DVE + ACT register sets).
  4. P^T = Wvo^T Xv^T + bvo (fp16 matmuls), written doubled along t from
     PSUM (ACT), per m-tile.
  5. out^T[d, t] = sum_i w_i * P^T[d, t + d_i] via runtime-register dynamic
     slices: 4x-mode tensor_scalar scales (DVE, taps 1-2 on ACT) + 2x
     tensor_tensor adds, pipelined per m-tile against the P-projection.
     Host transposes back and upcasts fp16 -> f32.

Measured (loop-differenced, 8 cores): 702us baseline -> 572us.  Rejected by
measurement: Pool-engine merge adds in the combine (cross-engine semaphore
serialization, +90us), pre-emitting P-proj m-tiles before the colsum, and
DMA-based P^T doubling (both slightly negative).  fp8 Gram is numerically
unsafe: the rank-8/9 correlation gap is ~0.0025 sigma_c and fp8 input
quantization noise (~5% sigma_c) would flip top-k taps (output error ~0.5).

Timing support: build_nc(kiter=K) emits the body K times separated by
all-engine barriers, so test.py can measure the marginal per-iteration
hardware execution time ((t_K - t_1) / (K - 1)) with dispatch overhead
cancelled.
"""
import os
import sys

if "/opt/trn_rl_repo" not in sys.path:
    sys.path.insert(0, "/opt/trn_rl_repo")

import numpy as np

import concourse.bacc as bacc
import concourse.mybir as mybir
import concourse.tile as tile
from concourse.bass import ds
from concourse.bass_types import AP
from concourse.masks import make_identity

B, L, D = 8, 3072, 1024
NT = L // 128          # 24 t-blocks
NC = L // 512          # 6 t-chunks
KT = D // 128          # 8 contraction tiles
MT = D // 128          # 8 output-channel tiles
TOPK = 8
N_CORES = 8
WG = 3200              # ring width incl prepended block (25*128)
WS = WG + 127          # skew row width

F32 = mybir.dt.float32
F16 = mybir.dt.float16
U32 = mybir.dt.uint32
AF = mybir.ActivationFunctionType
ALU = mybir.AluOpType

# row offsets of q/k/v in xpack, and of Wqk/Wvo in wpack.
# Weight folding: diag-sums of Q K^T equal diag-sums of Xq (Wq Wk^T) Xk^T up
# to a delay-independent constant (bias terms), which softmax and top-k both
# ignore; and the output is sum_i w_i shift_{d_i}(Xv (Wv Wo)) + (bv Wo + bo)
# because the delay aggregation commutes with the channel projection and
# sum_i w_i = 1.  So only TWO projections remain: Q' = Xq Wqk and
# P = Xv Wvo + bvo.
XOFF = {"q": 0, "k": 1, "v": 2}
WOFF = {"qk": 0, "vo": 1}


def build_nc(kiter=1):
    nc = bacc.Bacc("TRN2", target_bir_lowering=False, debug=False,
                   num_devices=N_CORES)

    aps = {
        "xpack": nc.dram_tensor("xpack", [3 * L, D], F16,
                                kind="ExternalInput").ap(),
        "wpack": nc.dram_tensor("wpack", [2 * D, D], F16,
                                kind="ExternalInput").ap(),
        "bpack": nc.dram_tensor("bpack", [1, D], F32,
                                kind="ExternalInput").ap(),
    }
    out = nc.dram_tensor("out", [D, L], F16, kind="ExternalOutput").ap()
    skew = nc.dram_tensor("skew", [128 * WS + 256], F32)
    with tile.TileContext(nc) as tc:
        for it in range(kiter):
            _kernel_body(tc, nc, aps, out, skew, itag=str(it))
            if it < kiter - 1:
                tc.strict_bb_all_engine_barrier()
    nc.compile()
    return nc


def _load_weights16(nc, pool, w_dram, tag):
    """W [din, dout] fp16 -> SBUF fp16 [128, KT*D]; w16[p, kt*D+n] = W[kt*128+p, n]."""
    w16 = pool.tile([128, KT * D], F16, tag="w16", name=f"w16_{tag}")
    nc.sync.dma_start(w16.rearrange("p (a n) -> p a n", a=KT),
                      w_dram.rearrange("(a p) n -> p a n", p=128))
    return w16


def _transpose_chunk_dma(nc, x_dram, x_base, c, xtp):
    """XBAR DMA-transpose fp16 x rows [512c, 512(c+1)) straight from DRAM into
    xtp [128, KT*512] with xtp[p, kt*512 + j] = x[x_base + 512c + j, kt*128+p].

    KTRSPLIT=1 alternates issues between the SP and ACT HWDGE queues —
    measured INCORRECT output (ACT-issued transpose XBAR DMAs corrupt the
    result), so it stays off."""
    split = int(os.environ.get("KTRSPLIT", "0"))
    for kt in range(KT):
        eng = nc.scalar if (split and kt % 2) else nc.sync
        eng.dma_start_transpose(
            xtp[:, 512 * kt:512 * (kt + 1)],
            x_dram[x_base + 512 * c: x_base + 512 * (c + 1),
                   128 * kt:128 * (kt + 1)])


def _transpose_chunk(nc, ident, x_dram, x_base, c, xin_pool, tpsum_pool, xtp,
                     itag):
    """PE-transpose fp16 x rows [512c, 512(c+1)) into xtp [128, KT*512] with
    xtp[p, kt*512 + al*128 + j] = x[x_base + 512c + al*128 + j, kt*128 + p]."""
    for al in range(4):
        a = 4 * c + al
        x16 = xin_pool.tile([128, D], F16, tag="x16",
                            name=f"x16_{c}_{al}_{itag}")
        nc.sync.dma_start(x16, x_dram[x_base + 128 * a:x_base + 128 * (a + 1), :])
        for half in range(2):
            pt = tpsum_pool.tile([128, 512], F16, tag="tp",
                                 name=f"pt_{c}_{al}_{half}_{itag}")
            for k2 in range(4):
                dt = 4 * half + k2
                nc.tensor.transpose(
                    pt[:, 128 * k2:128 * (k2 + 1)],
                    x16[:, 128 * dt:128 * (dt + 1)],
                    ident,
                )
            dst = xtp.rearrange("p (k f) -> p k f", f=512)[
                :, 4 * half:4 * half + 4, 128 * al:128 * (al + 1)]
            src = pt.rearrange("p (k f) -> p k f", f=128)
            nc.vector.tensor_copy(dst, src)


def _load_bias(nc, pool, b_dram, tag):
    """bias [1, D] f32 -> SBUF [128, MT]; b_sb[p, m] = bias[m*128+p]."""
    b_sb = pool.tile([128, MT], F32, tag=tag, name=f"b_{tag}")
    nc.sync.dma_start(b_sb, b_dram.rearrange("o (m p) -> (o p) m", p=128))
    return b_sb


def _kernel_body(tc, nc, aps, out, skew, itag="0"):
    import contextlib
    PHASES = int(os.environ.get("KPHASES", "9"))
    est = contextlib.ExitStack()

    xpack, wpack, bpack = aps["xpack"], aps["wpack"], aps["bpack"]

    bias_pool = est.enter_context(tc.tile_pool(name=f"bias{itag}", bufs=1))
    small_pool = est.enter_context(tc.tile_pool(name=f"small{itag}", bufs=1))
    kv_pool = est.enter_context(tc.tile_pool(name=f"kv{itag}", bufs=1))
    ring_pool = est.enter_context(tc.tile_pool(name=f"ring{itag}", bufs=1))
    est_kt = contextlib.ExitStack()
    kt_pool = est_kt.enter_context(tc.tile_pool(name=f"ktp{itag}", bufs=1))
    qt_pool = est_kt.enter_context(tc.tile_pool(name=f"qtp{itag}", bufs=1))

    b_sb = {"o": _load_bias(nc, bias_pool, bpack[0:1, :], f"bo_{itag}")}

    kt_sb = kt_pool.tile([128, MT * L], F16, tag="kt",
                         name=f"kt_sb_{itag}")    # Xk^T, m-major
    qt_sb = qt_pool.tile([128, MT * L], F16, tag="qt",
                         name=f"qt_sb_{itag}")    # Q'^T = (Xq Wqk)^T, m-major
    vt_sb = kv_pool.tile([128, MT * L], F16, tag="vt",
                         name=f"vt_sb_{itag}")    # Xv^T, m-major

    # ------- Phase 1: Xq^T transpose + Q' projection; Xk^T/Xv^T transpose ---
    TH = L // 3
    with tc.tile_pool(name=f"wpool{itag}", bufs=1) as wpool, \
         tc.tile_pool(name=f"xtp{itag}", bufs=2) as xtp_pool, \
         tc.tile_pool(name=f"ppsum{itag}", bufs=4, space="PSUM") as ppsum_pool:
        w16 = _load_weights16(nc, wpool,
                              wpack[WOFF["qk"] * D:(WOFF["qk"] + 1) * D, :],
                              f"wqk_{itag}")
        # Xq first (its transposes gate the only phase-1 PE work), then Xk
        # (gates the Gram), then Xv (only needed by the P-projection, which
        # runs after the Gram).  All XBAR transposes serialize on the shared
        # DMA/XBAR path (~43us per signal), so issue order = consumer order.
        x_base = XOFF["q"] * L
        for h in range(3):
            xtp = xtp_pool.tile([128, KT * TH], F16, tag="xtp3",
                                name=f"xtp3_q_{h}_{itag}")
            for kt in range(KT):
                nc.sync.dma_start_transpose(
                    xtp[:, TH * kt:TH * (kt + 1)],
                    xpack[x_base + TH * h: x_base + TH * (h + 1),
                          128 * kt:128 * (kt + 1)])
            for cc in range(2):
                c = 2 * h + cc
                for m in range(MT):
                    pp = ppsum_pool.tile([128, 512], F32, tag="pp",
                                         name=f"pp_q_{c}_{m}_{itag}")
                    for kt in range(KT):
                        nc.tensor.matmul(
                            pp,
                            w16[:, kt * D + 128 * m:
                                kt * D + 128 * (m + 1)],
                            xtp[:, TH * kt + 512 * cc:
                                TH * kt + 512 * (cc + 1)],
                            start=(kt == 0), stop=(kt == KT - 1),
                        )
                    nc.scalar.activation(
                        qt_sb[:, m * L + 512 * c: m * L + 512 * (c + 1)],
                        pp, AF.Identity, bias=0.0, scale=1.0)
        # Xk^T / Xv^T: straight XBAR DMA transposes into kt_sb / vt_sb
        # (no projection matmuls — the Gram consumes raw Xk^T and the
        # P-projection consumes raw Xv^T with the folded Wvo).
        for which, dst in (("k", kt_sb), ("v", vt_sb)):
            x_base = XOFF[which] * L
            for h in range(3):
                for kt in range(KT):
                    nc.sync.dma_start_transpose(
                        dst[:, kt * L + TH * h: kt * L + TH * (h + 1)],
                        xpack[x_base + TH * h: x_base + TH * (h + 1),
                              128 * kt:128 * (kt + 1)])

    if PHASES < 2:
        est_kt.close(); est.close(); return

    # ---------------- Phase 2: Gram + block-diagonal ring ----------------
    ring = ring_pool.tile([128, WG], F32, tag="ring", name=f"ring_{itag}")
    # KPSRING=1 accumulates the ring directly in PSUM (no DVE drains, exact
    # same math — needs the rotated a-loop because the matmul start bit
    # zeroes the whole target region).  Measured 606us vs 572us for the
    # default path: the +36% matmul instruction count (wrap splits) costs
    # more than the removed DVE drains, i.e. the drains were already hidden
    # and the Gram's overhead is per-matmul weight loads.  Kept off.
    if int(os.environ.get("KPSRING", "0")):
        # Accumulate the ring DIRECTLY in PSUM across a-tiles: ring column
        # rc (slot jj = rc//128 - 1, lane u = rc%128) needs
        #   sum_a sum_d qt[d, 128a+p] * kt[d, (128(a-1) + rc) % L],
        # i.e. for each (chunk, a, kt) one moving slice of Xk^T that is
        # contiguous except at the mod-L wrap (<= 2 matmul pieces).  This
        # removes all 144 DVE ring drains and their PSUM-reuse stalls; every
        # column's first/last contribution is at a=0 / a=NT-1, so start/stop
        # flags are uniform per a.
        with tc.tile_pool(name=f"gpsum{itag}", bufs=2,
                          space="PSUM") as gpsum_pool:
            for h in range((WG + 511) // 512):
                rc0 = 512 * h
                w = min(512, WG - rc0)
                rg = gpsum_pool.tile([128, w], F32, tag="rg",
                                     name=f"rg{h}_{itag}")
                # rotate the a-loop so the FIRST step is single-piece: the
                # start bit zeroes the whole target region, so a two-piece
                # first step would wipe its own first piece.
                a_start = next(a for a in range(NT)
                               if (128 * (a - 1) + rc0) % L + w <= L)
                for ai in range(NT):
                    a = (a_start + ai) % NT
                    for kt in range(KT):
                        t0 = (128 * (a - 1) + rc0) % L
                        if t0 + w <= L:
                            pieces = ((0, t0, w),)
                        else:
                            w1 = L - t0
                            pieces = ((0, t0, w1), (w1, 0, w - w1))
                        for po, ts, pwid in pieces:
                            nc.tensor.matmul(
                                rg[:, po:po + pwid],
                                qt_sb[:, kt * L + 128 * a:
                                      kt * L + 128 * (a + 1)],
                                kt_sb[:, kt * L + ts: kt * L + ts + pwid],
                                start=(ai == 0 and kt == 0),
                                stop=(ai == NT - 1 and kt == KT - 1),
                                skip_group_check=True,
                            )
                nc.vector.tensor_copy(ring[:, rc0:rc0 + w], rg)
        est_kt.close()  # K^T / Q'^T no longer needed
        if PHASES < 3:
            est.close(); return
        return _phase345(tc, nc, est, out, skew, ring, vt_sb, b_sb,
                         small_pool, wpack, itag)

    nc.vector.memset(ring, 0.0)
    with tc.tile_pool(name=f"gpsum{itag}", bufs=1, space="PSUM") as gpsum_pool:
        for a in range(NT):
            gps = [gpsum_pool.tile([128, 512], F32, tag=f"gp{c}",
                                   name=f"gp{a}_{c}_{itag}")
                   for c in range(NC)]
            if int(os.environ.get("KCMAJ", "1")):
                # c-major: each psum tile finishes early so its ring add
                # (DVE) overlaps the next tile's matmuls instead of
                # stalling the a+1 accumulation group on psum reuse.
                for c in range(NC):
                    for kt in range(KT):
                        nc.tensor.matmul(
                            gps[c],
                            qt_sb[:, kt * L + 128 * a:
                                  kt * L + 128 * (a + 1)],
                            kt_sb[:, kt * L + 512 * c:
                                  kt * L + 512 * (c + 1)],
                            start=(kt == 0), stop=(kt == KT - 1),
                        )
            else:
                for kt in range(KT):
                    for c in range(NC):
                        nc.tensor.matmul(
                            gps[c],
                            qt_sb[:, kt * L + 128 * a:
                                  kt * L + 128 * (a + 1)],
                            kt_sb[:, kt * L + 512 * c:
                                  kt * L + 512 * (c + 1)],
                            start=(kt == 0), stop=(kt == KT - 1),
                        )
            if int(os.environ.get("KNORING", "0")):
                # timing-only diagnostic: drop the ring drains entirely to
                # measure the pure Gram matmul stream (output is garbage)
                continue
            RD = int(os.environ.get("KRDRAIN", "0"))

            def radd(dst, in1, gp_slice, which):
                # KRDRAIN: split every drain column-wise between DVE and
                # Pool.  The two engines touch disjoint column ranges, so
                # each engine's RAW chain on `ring` stays engine-local (no
                # cross-engine semaphores on the accumulation path).
                if not RD:
                    nc.vector.tensor_add(dst, in1, gp_slice)
                    return
                n = dst.shape[-1]
                h = (n // 2) // 128 * 128 or n
                nc.vector.tensor_add(dst[:, :h], in1[:, :h], gp_slice[:, :h])
                if h < n:
                    nc.gpsimd.tensor_add(dst[:, h:], in1[:, h:],
                                         gp_slice[:, h:])

            for c in range(NC):
                gp = gps[c]
                jj0 = (4 * c - a) % NT
                off = 128 * (jj0 + 1)
                if jj0 <= NT - 4:
                    radd(ring[:, off:off + 512], ring[:, off:off + 512], gp,
                         c)
                else:
                    w1 = 128 * (NT - jj0)
                    radd(ring[:, off:off + w1], ring[:, off:off + w1],
                         gp[:, :w1], c)
                    radd(ring[:, 128:128 + 512 - w1],
                         ring[:, 128:128 + 512 - w1], gp[:, w1:], c)
    # ring block jj lives at offset 128*(jj+1); prepend a copy of block 23
    nc.vector.tensor_copy(ring[:, 0:128], ring[:, 128 * NT:128 * (NT + 1)])
    est_kt.close()  # K^T / Q'^T no longer needed
    if PHASES < 3:
        est.close(); return
    return _phase345(tc, nc, est, out, skew, ring, vt_sb, b_sb, small_pool,
                     wpack, itag)


def _phase345(tc, nc, est, out, skew, ring, vt_sb, b_sb, small_pool, wpack,
              itag):
    # ------ Phases 3-5: P-projection overlapped with skew -> top-8 ---------
    # The skew DMA round trip + colsum + max/softmax/register chain is a
    # serial ~25us tail that would idle the PE: emit the first KPRE m-tiles
    # of the P-projection BEFORE the colsum so the PE stays busy through it.
    PHASES = int(os.environ.get("KPHASES", "9"))
    PRE = max(0, min(int(os.environ.get("KPRE", "0")), MT))
    KDBL = int(os.environ.get("KDBLACT", "1"))
    with tc.tile_pool(name=f"wos{itag}", bufs=1) as wos_pool, \
         tc.tile_pool(name=f"p2tp{itag}", bufs=3) as p2t_pool, \
         tc.tile_pool(name=f"ppsum4{itag}", bufs=3, space="PSUM") as ppsum_pool, \
         tc.tile_pool(name=f"accp{itag}", bufs=2) as acc_pool, \
         tc.tile_pool(name=f"skp{itag}", bufs=1) as sk_pool, \
         tc.tile_pool(name=f"cspsum{itag}", bufs=1, space="PSUM") as cs_pool:
        wo16 = _load_weights16(nc, wos_pool,
                               wpack[WOFF["vo"] * D:(WOFF["vo"] + 1) * D, :],
                               f"wo_{itag}")

        def pproj(m):
            """P^T m-tile: matmuls + bias, doubled along t for the runtime
            circular slice (double via DMA on the otherwise idle SP queue
            unless KDBLACT=1 re-enables the ACT double-write)."""
            p2t = p2t_pool.tile([128, 2 * L], F16, tag="p2t",
                                name=f"p2t_{m}_{itag}")
            for c in range(NC):
                pp = ppsum_pool.tile([128, 512], F32, tag="pp",
                                     name=f"pp4_{c}_{m}_{itag}")
                for kt in range(KT):
                    nc.tensor.matmul(
                        pp,
                        wo16[:, kt * D + 128 * m: kt * D + 128 * (m + 1)],
                        vt_sb[:, kt * L + 512 * c: kt * L + 512 * (c + 1)],
                        start=(kt == 0), stop=(kt == KT - 1),
                    )
                nc.scalar.activation(
                    p2t[:, 512 * c: 512 * (c + 1)],
                    pp, AF.Identity, bias=b_sb["o"][:, m:m + 1], scale=1.0)
                if KDBL:
                    nc.scalar.activation(
                        p2t[:, L + 512 * c: L + 512 * (c + 1)],
                        pp, AF.Identity, bias=b_sb["o"][:, m:m + 1],
                        scale=1.0)
            if not KDBL:
                nc.sync.dma_start(p2t[:, L:2 * L], p2t[:, 0:L])
            return p2t

        p2ts = {}
        for m in range(PRE):
            p2ts[m] = pproj(m)

        # ---- skew -> colsum -> top-8 (DMA/PE-colsum/DVE under P-proj) ----
        sk_sb = sk_pool.tile([128, L], F32, tag="sk", name=f"sk_{itag}")
        skew_rd = AP(tensor=skew, offset=128, ap=[[WS, 128], [1, L]])
        skew_wr = AP(tensor=skew, offset=127, ap=[[WS - 1, 128], [1, WG]])
        nc.sync.dma_start(skew_wr, ring[:, 0:WG])    # skewed write
        nc.sync.dma_start(sk_sb, skew_rd)            # read back
        ones = sk_pool.tile([128, 1], F32, tag="ones", name=f"ones_{itag}")
        nc.vector.memset(ones, 1.0)
        # ring is dead once the skew write has read it — reuse its first row
        # as the colsum buffer (Tile orders the WAR hazard on the slice).
        colsum = ring[0:1, 0:L]
        for half in range(2):
            cs_psum = cs_pool.tile([1, L // 2], F32, tag="cs",
                                   name=f"cs_{half}_{itag}")
            for ch in range(NC // 2):
                nc.tensor.matmul(
                    cs_psum[:, 512 * ch:512 * (ch + 1)],
                    ones,
                    sk_sb[:, half * (L // 2) + 512 * ch:
                          half * (L // 2) + 512 * (ch + 1)],
                    start=True, stop=True,
                )
            nc.vector.tensor_copy(colsum[:, half * (L // 2):
                                         (half + 1) * (L // 2)], cs_psum)
        max8 = small_pool.tile([1, TOPK], F32, tag="max8", name=f"max8_{itag}")
        idx8 = small_pool.tile([1, TOPK], U32, tag="idx8", name=f"idx8_{itag}")
        sl = colsum[0:1, 0:L]
        nc.vector.max(out=max8, in_=sl)
        nc.vector.max_index(idx8, max8, sl)
        if PHASES < 4:
            est.close(); return

        # softmax(max8 / D)
        wts = small_pool.tile([1, TOPK], F32, tag="wts", name=f"wts_{itag}")
        negmax = small_pool.tile([1, 1], F32, tag="negmax",
                                 name=f"negmax_{itag}")
        inv = small_pool.tile([1, 1], F32, tag="inv", name=f"inv_{itag}")
        nc.vector.tensor_scalar_mul(negmax, max8[0:1, 0:1], -1.0 / D)
        nc.scalar.activation(wts, max8, AF.Exp, bias=negmax[0:1, 0:1],
                             scale=1.0 / D)
        nc.vector.reduce_sum(inv, wts, axis=mybir.AxisListType.X)
        nc.vector.reciprocal(inv, inv)
        nc.vector.tensor_scalar(wts, wts, inv[0:1, 0:1], None, op0=ALU.mult)
        w_bc = small_pool.tile([128, TOPK], F32, tag="wbc", name=f"wbc_{itag}")
        nc.gpsimd.partition_broadcast(w_bc, wts)

        # delay regs: m = idx; jd = m>>7; u = 127 - m%128; delta = (24-jd)%24;
        # d = 128*delta + u.  One register set per engine, only for the taps
        # that engine actually combines.
        engines = {"v": mybir.EngineType.DVE,
                   "a": mybir.EngineType.Activation}
        tap_sets = {"v": (0, 3, 4, 5, 6, 7), "a": (1, 2)}
        delay_sv = {}
        for key, etype in engines.items():
            eng = nc.engines[etype]
            svs = {}
            for i in tap_sets[key]:
                regs = nc.alloc_registers(f"dly{key}{i}i{itag}", (etype,))
                nc.regs_load(regs, idx8[0:1, i:i + 1])
                r0 = regs.handles[0]
                t1 = eng.alloc_register(f"t1{key}_{i}_{itag}")
                t2 = eng.alloc_register(f"t2{key}_{i}_{itag}")
                eng.reg_alu(t1, r0, 128, ALU.divide)      # jd
                eng.reg_alu(t2, t1, 128, ALU.mult)
                eng.reg_alu(r0, r0, t2, ALU.subtract)     # m % 128
                eng.reg_alu(r0, 127, r0, ALU.subtract)    # u
                eng.reg_alu(t1, NT, t1, ALU.subtract)     # 24 - jd
                eng.reg_alu(t1, t1, NT, ALU.mod)          # delta
                eng.reg_alu(t1, t1, 128, ALU.mult)
                eng.reg_alu(t1, t1, r0, ALU.add)          # d
                svs[i] = nc.snap(t1, min_val=0, max_val=L - 1)
            delay_sv[key] = svs

        # ---- remaining P-proj m-tiles + per-m combine (DVE/ACT/Pool) ----
        def combine(m, p2t):
            svs = delay_sv["v"]
            asvs = delay_sv["a"]
            acc = acc_pool.tile([128, L], F16, tag="acc", name=f"acc_{m}_{itag}")
            t_a = acc_pool.tile([128, L], F16, tag="t_a", name=f"ta_{m}_{itag}")
            t_b = acc_pool.tile([128, L], F16, tag="t_b", name=f"tb_{m}_{itag}")
            t4 = acc_pool.tile([128, L], F16, tag="t4", name=f"t4_{m}_{itag}")
            pw = p2t[:, 0:2 * L]
            # ACT: taps 1, 2 (activation scale)
            nc.scalar.activation(t_a, pw[:, ds(asvs[1], L)], AF.Identity,
                                 bias=0.0, scale=w_bc[:, 1:2])
            nc.scalar.activation(t_b, pw[:, ds(asvs[2], L)], AF.Identity,
                                 bias=0.0, scale=w_bc[:, 2:3])
            # DVE: all pointer-scaled taps (TensorScalarPtr is not legal on
            # Pool); taps 3,4 go to scratch tiles so the Pool engine can take
            # over 3 of the 7 merge adds (plain TensorTensor is Pool-legal).
            nc.vector.tensor_scalar(acc, pw[:, ds(svs[0], L)],
                                    w_bc[:, 0:1], None, op0=ALU.mult)
            for i in (5, 6, 7):
                nc.vector.tensor_scalar(t4, pw[:, ds(svs[i], L)],
                                        w_bc[:, i:i + 1], None, op0=ALU.mult)
                nc.vector.tensor_add(acc, acc, t4)
            if int(os.environ.get("KGPS", "0")):
                t3s = acc_pool.tile([128, L], F16, tag="t3s",
                                    name=f"t3s_{m}_{itag}")
                t4s = acc_pool.tile([128, L], F16, tag="t4s",
                                    name=f"t4s_{m}_{itag}")
                nc.vector.tensor_scalar(t3s, pw[:, ds(svs[3], L)],
                                        w_bc[:, 3:4], None, op0=ALU.mult)
                nc.vector.tensor_scalar(t4s, pw[:, ds(svs[4], L)],
                                        w_bc[:, 4:5], None, op0=ALU.mult)
                nc.gpsimd.tensor_add(t3s, t3s, t4s)
                nc.gpsimd.tensor_add(t3s, t3s, t_a)
                nc.gpsimd.tensor_add(t3s, t3s, t_b)
                nc.vector.tensor_add(acc, acc, t3s)
            else:
                for i in (3, 4):
                    nc.vector.tensor_scalar(t4, pw[:, ds(svs[i], L)],
                                            w_bc[:, i:i + 1], None,
                                            op0=ALU.mult)
                    nc.vector.tensor_add(acc, acc, t4)
                nc.vector.tensor_add(t_a, t_a, t_b)
                nc.vector.tensor_add(acc, acc, t_a)
            nc.sync.dma_start(out[128 * m:128 * (m + 1), :], acc)

        COMBM = int(os.environ.get("KCOMBM", "8"))
        for m in range(MT):
            p2t = p2ts[m] if m in p2ts else pproj(m)
            if m < COMBM:
                combine(m, p2t)

    est.close()


# ------------------------- host-side wrapper -------------------------
_CACHE = {}


def _build_runner(kiter=1, donate=True):
    """Build nc + a cached jitted SPMD callable (mirrors run_bass_via_pjrt).

    donate=False keeps the zero output buffers as ordinary (reusable) inputs:
    the kernel writes every element of `out`, so the pre-zeroed donation is
    only an XLA aliasing optimization, not a correctness requirement.  Timing
    harnesses use donate=False so staged device arrays can be reused across
    back-to-back dispatches."""
    import jax
    from jax.sharding import Mesh, PartitionSpec
    from jax.experimental.shard_map import shard_map
    from concourse import bass2jax
    import concourse.mybir as mb

    nc = build_nc(kiter=kiter)
    bass2jax.install_neuronx_cc_hook()

    partition_name = (nc.partition_id_tensor.name
                      if nc.partition_id_tensor else None)
    in_names, out_names, out_avals, zero_outs = [], [], [], []
    for alloc in nc.m.functions[0].allocations:
        if not isinstance(alloc, mb.MemoryLocationSet):
            continue
        name = alloc.memorylocations[0].name
        if alloc.kind == "ExternalInput":
            if name != partition_name:
                in_names.append(name)
        elif alloc.kind == "ExternalOutput":
            shape = tuple(alloc.tensor_shape)
            dtype = mb.dt.np(alloc.dtype)
            out_names.append(name)
            out_avals.append(jax.core.ShapedArray(shape, dtype))
            zero_outs.append(np.zeros(shape, dtype))
    n_params = len(in_names)
    all_names = list(in_names) + list(out_names)
    if partition_name is not None:
        all_names.append(partition_name)
    donate_nums = (tuple(range(n_params, n_params + len(out_names)))
                   if donate else ())

    def _body(*args):
        operands = list(args)
        if partition_name is not None:
            operands.append(bass2jax.partition_id_tensor())
        return tuple(bass2jax._bass_exec_p.bind(
            *operands,
            out_avals=tuple(out_avals),
            in_names=tuple(all_names),
            out_names=tuple(out_names),
            lowering_input_output_aliases=(),
            sim_require_finite=True,
            sim_require_nnan=True,
            nc=nc,
        ))

    devices = jax.devices()[:N_CORES]
    mesh = Mesh(np.asarray(devices), ("core",))
    in_specs = (PartitionSpec("core"),) * (n_params + len(out_names))
    out_specs = (PartitionSpec("core"),) * len(out_names)
    sharded = jax.jit(
        shard_map(_body, mesh=mesh, in_specs=in_specs, out_specs=out_specs,
                  check_rep=False),
        donate_argnums=donate_nums, keep_unused=True)
    return {
        "sharded": sharded, "in_names": in_names, "out_names": out_names,
        "out_avals": out_avals, "zero_outs": zero_outs,
    }


def _get_runner(kiter=1, donate=True):
    key = (kiter, donate)
    if key not in _CACHE:
        _CACHE[key] = _build_runner(kiter=kiter, donate=donate)
    return _CACHE[key]


def _concat_inputs(r, in_maps):
    per_core = [[np.asarray(m[name]) for name in r["in_names"]]
                for m in in_maps]
    concat_in = [np.concatenate([per_core[c][i] for c in range(N_CORES)],
                                axis=0)
                 for i in range(len(r["in_names"]))]
    concat_zeros = [np.zeros((N_CORES * z.shape[0], *z.shape[1:]), z.dtype)
                    for z in r["zero_outs"]]
    return concat_in, concat_zeros


def _run(r, concat_in, concat_zeros):
    out_arrs = r["sharded"](*concat_in, *concat_zeros)
    return [
        {name: np.asarray(out_arrs[i]).reshape(
            N_CORES, *r["out_avals"][i].shape)[c]
         for i, name in enumerate(r["out_names"])}
        for c in range(N_CORES)
    ]


def make_in_maps(queries, keys, values, Wq, bq, Wk, bk, Wv, bv, Wo, bo):
    """Pack full f32 inputs into per-core fp16 in_maps with folded weights.

    Wqk = Wq Wk^T: diag-sums of Q K^T equal diag-sums of Xq Wqk Xk^T up to a
    delay-independent constant (the bias cross-terms), which top-k ordering
    and softmax are both invariant to.  Wvo = Wv Wo (+ bvo = bv Wo + bo):
    the delay aggregation is a convex combination of time shifts and commutes
    with the channel projection, so V-proj and O-proj fuse into one matmul.
    """
    Wqk = (np.asarray(Wq, np.float64) @ np.asarray(Wk, np.float64).T)
    Wvo = (np.asarray(Wv, np.float64) @ np.asarray(Wo, np.float64))
    bvo = (np.asarray(bv, np.float64) @ np.asarray(Wo, np.float64)
           + np.asarray(bo, np.float64))
    wpack = np.concatenate([Wqk, Wvo], axis=0).astype(np.float16)
    bpack = bvo[None, :].astype(np.float32)
    queries = np.asarray(queries, np.float32)
    keys = np.asarray(keys, np.float32)
    values = np.asarray(values, np.float32)
    in_maps = []
    for b in range(B):
        xpack = np.concatenate(
            [queries[b], keys[b], values[b]], axis=0).astype(np.float16)
        in_maps.append({"xpack": xpack, "wpack": wpack, "bpack": bpack})
    return in_maps


def kernel(queries, keys, values, Wq, bq, Wk, bk, Wv, bv, Wo, bo):
    r = _get_runner(kiter=1)
    in_maps = make_in_maps(queries, keys, values, Wq, bq, Wk, bk, Wv, bv,
                           Wo, bo)
    concat_in, concat_zeros = _concat_inputs(r, in_maps)
    results = _run(r, concat_in, concat_zeros)
    outs = [results[b]["out"].T.astype(np.float32) for b in range(B)]
    return np.ascontiguousarray(np.stack(outs))


if __name__ == "__main__":
    rng = np.random.default_rng(0)
    ins = {
        "queries": rng.standard_normal((B, L, D)).astype(np.float32),
        "keys": rng.standard_normal((B, L, D)).astype(np.float32),
        "values": rng.standard_normal((B, L, D)).astype(np.float32),
        "Wq": (rng.standard_normal((D, D)) * 0.02).astype(np.float32),
        "bq": np.zeros(D, np.float32),
        "Wk": (rng.standard_normal((D, D)) * 0.02).astype(np.float32),
        "bk": np.zeros(D, np.float32),
        "Wv": (rng.standard_normal((D, D)) * 0.02).astype(np.float32),
        "bv": np.zeros(D, np.float32),
        "Wo": (rng.standard_normal((D, D)) * 0.02).astype(np.float32),
        "bo": np.zeros(D, np.float32),
    }
    o = kernel(**ins)
    print("out", o.shape, o.dtype, float(np.abs(o).max()))



# revision 26
# speedup vs baseline: 1.0603x; 1.0191x over previous
"""Trainium2 Bass kernel for the Autoformer autocorrelation block.

Contract: kernel(**inputs) takes FULL inputs (B=8 batches), returns FULL output
[8, 3072, 1024] f32. Internally: data-parallel over batch across 8 NeuronCores.

Weight folding (host side, fp64): the correlation only needs circular
diag-sums of Q K^T = Xq (Wq Wk^T) Xk^T + bias terms that are constant in the
delay, and top-k + softmax are invariant to constant shifts — so Wqk = Wq Wk^T
replaces the Q AND K projections with a single one.  The delay aggregation is
a convex combination of time shifts and commutes with the channel projection,
so Wvo = Wv Wo (bvo = bv Wo + bo) fuses the V and O projections.  PE work
drops from 5 to 2 projections + the Gram (22.5 -> 16.1 GMAC per core).

Per-core algorithm (one batch; t = time in [0,3072), d = channel in [0,1024)):
  1. XBAR DMA-transpose Xq (thirds, pipelined) and project with fp16 Wqk on
     the PE -> Q'^T in [d, t] layout; Xk^T and Xv^T are straight XBAR DMA
     transposes (no matmuls).  Issue order q -> k -> v matches consumer
     order since all transposes serialize on the shared XBAR path.
  2. diag-sums via Gram tiles on PE with block-diagonal ring accumulation
     (ring[jj] = sum of [128,128] blocks with (b-a)%24 == jj), then a
     strided-DMA "skew" through DRAM turns diagonals into columns and a
     PE ones-matmul reduce yields all 3072 diagonal sums at once (the dead
     ring tile's first row is reused as the colsum buffer).
  3. top-8 values+indices via DVE max/max_index; softmax on-device; delay
     values recovered with register ALU (DVE + ACT register sets).
  4. P^T = Wvo^T Xv^T + bvo (fp16 matmuls), written doubled along t from
     PSUM (ACT), per m-tile.
  5. out^T[d, t] = sum_i w_i * P^T[d, t + d_i] via runtime-register dynamic
     slices: 4x-mode tensor_scalar scales (DVE, taps 1-2 on ACT) + 2x
     tensor_tensor adds, pipelined per m-tile against the P-projection.
     Host transposes back and upcasts fp16 -> f32.

Measured (loop-differenced, 8 cores): 702us baseline -> 572us.  Rejected by
measurement: Pool-engine merge adds in the combine (cross-engine semaphore
serialization, +90us), pre-emitting P-proj m-tiles before the colsum, and
DMA-based P^T doubling (both slightly negative).  fp8 Gram is numerically
unsafe: the rank-8/9 correlation gap is ~0.0025 sigma_c and fp8 input
quantization noise (~5% sigma_c) would flip top-k taps (output error ~0.5).

Timing support: build_nc(kiter=K) emits the body K times separated by
all-engine barriers, so test.py can measure the marginal per-iteration
hardware execution time ((t_K - t_1) / (K - 1)) with dispatch overhead
cancelled.
"""
import os
import sys

if "/opt/trn_rl_repo" not in sys.path:
    sys.path.insert(0, "/opt/trn_rl_repo")

import numpy as np

import concourse.bacc as bacc
import concourse.mybir as mybir
import concourse.tile as tile
from concourse.bass import ds
from concourse.bass_types import AP
from concourse.masks import make_identity

B, L, D = 8, 3072, 1024
NT = L // 128          # 24 t-blocks
NC = L // 512          # 6 t-chunks
KT = D // 128          # 8 contraction tiles
MT = D // 128          # 8 output-channel tiles
TOPK = 8
N_CORES = 8
WG = 3200              # ring width incl prepended block (25*128)
WS = WG + 127          # skew row width

F32 = mybir.dt.float32
F16 = mybir.dt.float16
U32 = mybir.dt.uint32
AF = mybir.ActivationFunctionType
ALU = mybir.AluOpType

# row offsets of q/k/v in xpack, and of Wqk/Wvo in wpack.
# Weight folding: diag-sums of Q K^T equal diag-sums of Xq (Wq Wk^T) Xk^T up
# to a delay-independent constant (bias terms), which softmax and top-k both
# ignore; and the output is sum_i w_i shift_{d_i}(Xv (Wv Wo)) + (bv Wo + bo)
# because the delay aggregation commutes with the channel projection and
# sum_i w_i = 1.  So only TWO projections remain: Q' = Xq Wqk and
# P = Xv Wvo + bvo.
XOFF = {"q": 0, "k": 1, "v": 2}
WOFF = {"qk": 0, "vo": 1}


def build_nc(kiter=1):
    nc = bacc.Bacc("TRN2", target_bir_lowering=False, debug=False,
                   num_devices=N_CORES)

    aps = {
        "xpack": nc.dram_tensor("xpack", [3 * L, D], F16,
                                kind="ExternalInput").ap(),
        "wpack": nc.dram_tensor("wpack", [2 * D, D], F16,
                                kind="ExternalInput").ap(),
        "bpack": nc.dram_tensor("bpack", [1, D], F32,
                                kind="ExternalInput").ap(),
    }
    out = nc.dram_tensor("out", [D, L], F16, kind="ExternalOutput").ap()
    skew = nc.dram_tensor("skew", [128 * WS + 256], F32)
    with tile.TileContext(nc) as tc:
        for it in range(kiter):
            _kernel_body(tc, nc, aps, out, skew, itag=str(it))
            if it < kiter - 1:
                tc.strict_bb_all_engine_barrier()
    nc.compile()
    return nc


def _load_weights16(nc, pool, w_dram, tag):
    """W [din, dout] fp16 -> SBUF fp16 [128, KT*D]; w16[p, kt*D+n] = W[kt*128+p, n]."""
    w16 = pool.tile([128, KT * D], F16, tag="w16", name=f"w16_{tag}")
    nc.sync.dma_start(w16.rearrange("p (a n) -> p a n", a=KT),
                      w_dram.rearrange("(a p) n -> p a n", p=128))
    return w16


def _transpose_chunk_dma(nc, x_dram, x_base, c, xtp):
    """XBAR DMA-transpose fp16 x rows [512c, 512(c+1)) straight from DRAM into
    xtp [128, KT*512] with xtp[p, kt*512 + j] = x[x_base + 512c + j, kt*128+p].

    KTRSPLIT=1 alternates issues between the SP and ACT HWDGE queues —
    measured INCORRECT output (ACT-issued transpose XBAR DMAs corrupt the
    result), so it stays off."""
    split = int(os.environ.get("KTRSPLIT", "0"))
    for kt in range(KT):
        eng = nc.scalar if (split and kt % 2) else nc.sync
        eng.dma_start_transpose(
            xtp[:, 512 * kt:512 * (kt + 1)],
            x_dram[x_base + 512 * c: x_base + 512 * (c + 1),
                   128 * kt:128 * (kt + 1)])


def _transpose_chunk(nc, ident, x_dram, x_base, c, xin_pool, tpsum_pool, xtp,
                     itag):
    """PE-transpose fp16 x rows [512c, 512(c+1)) into xtp [128, KT*512] with
    xtp[p, kt*512 + al*128 + j] = x[x_base + 512c + al*128 + j, kt*128 + p]."""
    for al in range(4):
        a = 4 * c + al
        x16 = xin_pool.tile([128, D], F16, tag="x16",
                            name=f"x16_{c}_{al}_{itag}")
        nc.sync.dma_start(x16, x_dram[x_base + 128 * a:x_base + 128 * (a + 1), :])
        for half in range(2):
            pt = tpsum_pool.tile([128, 512], F16, tag="tp",
                                 name=f"pt_{c}_{al}_{half}_{itag}")
            for k2 in range(4):
                dt = 4 * half + k2
                nc.tensor.transpose(
                    pt[:, 128 * k2:128 * (k2 + 1)],
                    x16[:, 128 * dt:128 * (dt + 1)],
                    ident,
                )
            dst = xtp.rearrange("p (k f) -> p k f", f=512)[
                :, 4 * half:4 * half + 4, 128 * al:128 * (al + 1)]
            src = pt.rearrange("p (k f) -> p k f", f=128)
            nc.vector.tensor_copy(dst, src)


def _load_bias(nc, pool, b_dram, tag):
    """bias [1, D] f32 -> SBUF [128, MT]; b_sb[p, m] = bias[m*128+p]."""
    b_sb = pool.tile([128, MT], F32, tag=tag, name=f"b_{tag}")
    nc.sync.dma_start(b_sb, b_dram.rearrange("o (m p) -> (o p) m", p=128))
    return b_sb


def _kernel_body(tc, nc, aps, out, skew, itag="0"):
    import contextlib
    PHASES = int(os.environ.get("KPHASES", "9"))
    est = contextlib.ExitStack()

    xpack, wpack, bpack = aps["xpack"], aps["wpack"], aps["bpack"]

    bias_pool = est.enter_context(tc.tile_pool(name=f"bias{itag}", bufs=1))
    small_pool = est.enter_context(tc.tile_pool(name=f"small{itag}", bufs=1))
    kv_pool = est.enter_context(tc.tile_pool(name=f"kv{itag}", bufs=1))
    ring_pool = est.enter_context(tc.tile_pool(name=f"ring{itag}", bufs=1))
    est_kt = contextlib.ExitStack()
    kt_pool = est_kt.enter_context(tc.tile_pool(name=f"ktp{itag}", bufs=1))
    qt_pool = est_kt.enter_context(tc.tile_pool(name=f"qtp{itag}", bufs=1))

    b_sb = {"o": _load_bias(nc, bias_pool, bpack[0:1, :], f"bo_{itag}")}

    kt_sb = kt_pool.tile([128, MT * L], F16, tag="kt",
                         name=f"kt_sb_{itag}")    # Xk^T, m-major
    qt_sb = qt_pool.tile([128, MT * L], F16, tag="qt",
                         name=f"qt_sb_{itag}")    # Q'^T = (Xq Wqk)^T, m-major
    vt_sb = kv_pool.tile([128, MT * L], F16, tag="vt",
                         name=f"vt_sb_{itag}")    # Xv^T, m-major

    # ------- Phase 1: Xq^T transpose + Q' projection; Xk^T/Xv^T transpose ---
    TH = L // 3
    with tc.tile_pool(name=f"wpool{itag}", bufs=1) as wpool, \
         tc.tile_pool(name=f"xtp{itag}", bufs=2) as xtp_pool, \
         tc.tile_pool(name=f"ppsum{itag}", bufs=4, space="PSUM") as ppsum_pool:
        w16 = _load_weights16(nc, wpool,
                              wpack[WOFF["qk"] * D:(WOFF["qk"] + 1) * D, :],
                              f"wqk_{itag}")
        # Xq first (its transposes gate the only phase-1 PE work), then Xk
        # (gates the Gram), then Xv (only needed by the P-projection, which
        # runs after the Gram).  All XBAR transposes serialize on the shared
        # DMA/XBAR path (~43us per signal), so issue order = consumer order.
        x_base = XOFF["q"] * L
        for h in range(3):
            xtp = xtp_pool.tile([128, KT * TH], F16, tag="xtp3",
                                name=f"xtp3_q_{h}_{itag}")
            for kt in range(KT):
                nc.sync.dma_start_transpose(
                    xtp[:, TH * kt:TH * (kt + 1)],
                    xpack[x_base + TH * h: x_base + TH * (h + 1),
                          128 * kt:128 * (kt + 1)])
            for cc in range(2):
                c = 2 * h + cc
                for m in range(MT):
                    pp = ppsum_pool.tile([128, 512], F32, tag="pp",
                                         name=f"pp_q_{c}_{m}_{itag}")
                    for kt in range(KT):
                        nc.tensor.matmul(
                            pp,
                            w16[:, kt * D + 128 * m:
                                kt * D + 128 * (m + 1)],
                            xtp[:, TH * kt + 512 * cc:
                                TH * kt + 512 * (cc + 1)],
                            start=(kt == 0), stop=(kt == KT - 1),
                        )
                    nc.scalar.activation(
                        qt_sb[:, m * L + 512 * c: m * L + 512 * (c + 1)],
                        pp, AF.Identity, bias=0.0, scale=1.0)
        # Xk^T / Xv^T: straight XBAR DMA transposes into kt_sb / vt_sb
        # (no projection matmuls — the Gram consumes raw Xk^T and the
        # P-projection consumes raw Xv^T with the folded Wvo).
        for which, dst in (("k", kt_sb), ("v", vt_sb)):
            x_base = XOFF[which] * L
            for h in range(3):
                for kt in range(KT):
                    nc.sync.dma_start_transpose(
                        dst[:, kt * L + TH * h: kt * L + TH * (h + 1)],
                        xpack[x_base + TH * h: x_base + TH * (h + 1),
                              128 * kt:128 * (kt + 1)])

    if PHASES < 2:
        est_kt.close(); est.close(); return

    # ---------------- Phase 2: Gram + block-diagonal ring ----------------
    ring = ring_pool.tile([128, WG], F32, tag="ring", name=f"ring_{itag}")
    # KPSRING=1 accumulates the ring directly in PSUM (no DVE drains, exact
    # same math — needs the rotated a-loop because the matmul start bit
    # zeroes the whole target region).  Measured 606us vs 572us for the
    # default path: the +36% matmul instruction count (wrap splits) costs
    # more than the removed DVE drains, i.e. the drains were already hidden
    # and the Gram's overhead is per-matmul weight loads.  Kept off.
    if int(os.environ.get("KPSRING", "0")):
        # Accumulate the ring DIRECTLY in PSUM across a-tiles: ring column
        # rc (slot jj = rc//128 - 1, lane u = rc%128) needs
        #   sum_a sum_d qt[d, 128a+p] * kt[d, (128(a-1) + rc) % L],
        # i.e. for each (chunk, a, kt) one moving slice of Xk^T that is
        # contiguous except at the mod-L wrap (<= 2 matmul pieces).  This
        # removes all 144 DVE ring drains and their PSUM-reuse stalls; every
        # column's first/last contribution is at a=0 / a=NT-1, so start/stop
        # flags are uniform per a.
        with tc.tile_pool(name=f"gpsum{itag}", bufs=2,
                          space="PSUM") as gpsum_pool:
            for h in range((WG + 511) // 512):
                rc0 = 512 * h
                w = min(512, WG - rc0)
                rg = gpsum_pool.tile([128, w], F32, tag="rg",
                                     name=f"rg{h}_{itag}")
                # rotate the a-loop so the FIRST step is single-piece: the
                # start bit zeroes the whole target region, so a two-piece
                # first step would wipe its own first piece.
                a_start = next(a for a in range(NT)
                               if (128 * (a - 1) + rc0) % L + w <= L)
                for ai in range(NT):
                    a = (a_start + ai) % NT
                    for kt in range(KT):
                        t0 = (128 * (a - 1) + rc0) % L
                        if t0 + w <= L:
                            pieces = ((0, t0, w),)
                        else:
                            w1 = L - t0
                            pieces = ((0, t0, w1), (w1, 0, w - w1))
                        for po, ts, pwid in pieces:
                            nc.tensor.matmul(
                                rg[:, po:po + pwid],
                                qt_sb[:, kt * L + 128 * a:
                                      kt * L + 128 * (a + 1)],
                                kt_sb[:, kt * L + ts: kt * L + ts + pwid],
                                start=(ai == 0 and kt == 0),
                                stop=(ai == NT - 1 and kt == KT - 1),
                                skip_group_check=True,
                            )
                nc.vector.tensor_copy(ring[:, rc0:rc0 + w], rg)
        est_kt.close()  # K^T / Q'^T no longer needed
        if PHASES < 3:
            est.close(); return
        return _phase345(tc, nc, est, out, skew, ring, vt_sb, b_sb,
                         small_pool, wpack, itag)

    nc.vector.memset(ring, 0.0)
    with tc.tile_pool(name=f"gpsum{itag}", bufs=1, space="PSUM") as gpsum_pool:
        for a in range(NT):
            gps = [gpsum_pool.tile([128, 512], F32, tag=f"gp{c}",
                                   name=f"gp{a}_{c}_{itag}")
                   for c in range(NC)]
            if int(os.environ.get("KCMAJ", "1")):
                # c-major: each psum tile finishes early so its ring add
                # (DVE) overlaps the next tile's matmuls instead of
                # stalling the a+1 accumulation group on psum reuse.
                for c in range(NC):
                    for kt in range(KT):
                        nc.tensor.matmul(
                            gps[c],
                            qt_sb[:, kt * L + 128 * a:
                                  kt * L + 128 * (a + 1)],
                            kt_sb[:, kt * L + 512 * c:
                                  kt * L + 512 * (c + 1)],
                            start=(kt == 0), stop=(kt == KT - 1),
                        )
            else:
                for kt in range(KT):
                    for c in range(NC):
                        nc.tensor.matmul(
                            gps[c],
                            qt_sb[:, kt * L + 128 * a:
                                  kt * L + 128 * (a + 1)],
                            kt_sb[:, kt * L + 512 * c:
                                  kt * L + 512 * (c + 1)],
                            start=(kt == 0), stop=(kt == KT - 1),
                        )
            if int(os.environ.get("KNORING", "0")):
                # timing-only diagnostic: drop the ring drains entirely to
                # measure the pure Gram matmul stream (output is garbage)
                continue
            RD = int(os.environ.get("KRDRAIN", "0"))

            def radd(dst, in1, gp_slice, which):
                # KRDRAIN: split every drain column-wise between DVE and
                # Pool.  The two engines touch disjoint column ranges, so
                # each engine's RAW chain on `ring` stays engine-local (no
                # cross-engine semaphores on the accumulation path).
                if not RD:
                    nc.vector.tensor_add(dst, in1, gp_slice)
                    return
                n = dst.shape[-1]
                h = (n // 2) // 128 * 128 or n
                nc.vector.tensor_add(dst[:, :h], in1[:, :h], gp_slice[:, :h])
                if h < n:
                    nc.gpsimd.tensor_add(dst[:, h:], in1[:, h:],
                                         gp_slice[:, h:])

            for c in range(NC):
                gp = gps[c]
                jj0 = (4 * c - a) % NT
                off = 128 * (jj0 + 1)
                if jj0 <= NT - 4:
                    radd(ring[:, off:off + 512], ring[:, off:off + 512], gp,
                         c)
                else:
                    w1 = 128 * (NT - jj0)
                    radd(ring[:, off:off + w1], ring[:, off:off + w1],
                         gp[:, :w1], c)
                    radd(ring[:, 128:128 + 512 - w1],
                         ring[:, 128:128 + 512 - w1], gp[:, w1:], c)
    # ring block jj lives at offset 128*(jj+1); prepend a copy of block 23
    nc.vector.tensor_copy(ring[:, 0:128], ring[:, 128 * NT:128 * (NT + 1)])
    est_kt.close()  # K^T / Q'^T no longer needed
    if PHASES < 3:
        est.close(); return
    return _phase345(tc, nc, est, out, skew, ring, vt_sb, b_sb, small_pool,
                     wpack, itag)


def _phase345(tc, nc, est, out, skew, ring, vt_sb, b_sb, small_pool, wpack,
              itag):
    # ------ Phases 3-5: P-projection overlapped with skew -> top-8 ---------
    # The skew DMA round trip + colsum + max/softmax/register chain is a
    # serial ~25us tail that would idle the PE: emit the first KPRE m-tiles
    # of the P-projection BEFORE the colsum so the PE stays busy through it.
    PHASES = int(os.environ.get("KPHASES", "9"))
    PRE = max(0, min(int(os.environ.get("KPRE", "0")), MT))
    KDBL = int(os.environ.get("KDBLACT", "1"))
    with tc.tile_pool(name=f"wos{itag}", bufs=1) as wos_pool, \
         tc.tile_pool(name=f"p2tp{itag}", bufs=3) as p2t_pool, \
         tc.tile_pool(name=f"ppsum4{itag}", bufs=2, space="PSUM") as ppsum_pool, \
         tc.tile_pool(name=f"pcpsum{itag}", bufs=2, space="PSUM") as pc_pool, \
         tc.tile_pool(name=f"accp{itag}", bufs=2) as acc_pool, \
         tc.tile_pool(name=f"skp{itag}", bufs=1) as sk_pool, \
         tc.tile_pool(name=f"cspsum{itag}", bufs=1, space="PSUM") as cs_pool:
        wo16 = _load_weights16(nc, wos_pool,
                               wpack[WOFF["vo"] * D:(WOFF["vo"] + 1) * D, :],
                               f"wo_{itag}")

        def pproj(m):
            """P^T m-tile: matmuls + bias, doubled along t for the runtime
            circular slice (double via DMA on the otherwise idle SP queue
            unless KDBLACT=1 re-enables the ACT double-write)."""
            p2t = p2t_pool.tile([128, 2 * L], F16, tag="p2t",
                                name=f"p2t_{m}_{itag}")
            for c in range(NC):
                pp = ppsum_pool.tile([128, 512], F32, tag="pp",
                                     name=f"pp4_{c}_{m}_{itag}")
                for kt in range(KT):
                    nc.tensor.matmul(
                        pp,
                        wo16[:, kt * D + 128 * m: kt * D + 128 * (m + 1)],
                        vt_sb[:, kt * L + 512 * c: kt * L + 512 * (c + 1)],
                        start=(kt == 0), stop=(kt == KT - 1),
                    )
                nc.scalar.activation(
                    p2t[:, 512 * c: 512 * (c + 1)],
                    pp, AF.Identity, bias=b_sb["o"][:, m:m + 1], scale=1.0)
                if KDBL:
                    nc.scalar.activation(
                        p2t[:, L + 512 * c: L + 512 * (c + 1)],
                        pp, AF.Identity, bias=b_sb["o"][:, m:m + 1],
                        scale=1.0)
            if not KDBL:
                nc.sync.dma_start(p2t[:, L:2 * L], p2t[:, 0:L])
            return p2t

        p2ts = {}
        for m in range(PRE):
            p2ts[m] = pproj(m)

        # ---- skew -> colsum -> top-8 (DMA/PE-colsum/DVE under P-proj) ----
        sk_sb = sk_pool.tile([128, L], F32, tag="sk", name=f"sk_{itag}")
        skew_rd = AP(tensor=skew, offset=128, ap=[[WS, 128], [1, L]])
        skew_wr = AP(tensor=skew, offset=127, ap=[[WS - 1, 128], [1, WG]])
        nc.sync.dma_start(skew_wr, ring[:, 0:WG])    # skewed write
        nc.sync.dma_start(sk_sb, skew_rd)            # read back
        ones = sk_pool.tile([128, 1], F32, tag="ones", name=f"ones_{itag}")
        nc.vector.memset(ones, 1.0)
        # ring is dead once the skew write has read it — reuse its first row
        # as the colsum buffer (Tile orders the WAR hazard on the slice).
        colsum = ring[0:1, 0:L]
        for half in range(2):
            cs_psum = cs_pool.tile([1, L // 2], F32, tag="cs",
                                   name=f"cs_{half}_{itag}")
            for ch in range(NC // 2):
                nc.tensor.matmul(
                    cs_psum[:, 512 * ch:512 * (ch + 1)],
                    ones,
                    sk_sb[:, half * (L // 2) + 512 * ch:
                          half * (L // 2) + 512 * (ch + 1)],
                    start=True, stop=True,
                )
            nc.vector.tensor_copy(colsum[:, half * (L // 2):
                                         (half + 1) * (L // 2)], cs_psum)
        max8 = small_pool.tile([1, TOPK], F32, tag="max8", name=f"max8_{itag}")
        idx8 = small_pool.tile([1, TOPK], U32, tag="idx8", name=f"idx8_{itag}")
        sl = colsum[0:1, 0:L]
        nc.vector.max(out=max8, in_=sl)
        nc.vector.max_index(idx8, max8, sl)
        if PHASES < 4:
            est.close(); return

        # softmax(max8 / D)
        wts = small_pool.tile([1, TOPK], F32, tag="wts", name=f"wts_{itag}")
        negmax = small_pool.tile([1, 1], F32, tag="negmax",
                                 name=f"negmax_{itag}")
        inv = small_pool.tile([1, 1], F32, tag="inv", name=f"inv_{itag}")
        nc.vector.tensor_scalar_mul(negmax, max8[0:1, 0:1], -1.0 / D)
        nc.scalar.activation(wts, max8, AF.Exp, bias=negmax[0:1, 0:1],
                             scale=1.0 / D)
        nc.vector.reduce_sum(inv, wts, axis=mybir.AxisListType.X)
        nc.vector.reciprocal(inv, inv)
        nc.vector.tensor_scalar(wts, wts, inv[0:1, 0:1], None, op0=ALU.mult)
        w_bc = small_pool.tile([128, TOPK], F32, tag="wbc", name=f"wbc_{itag}")
        nc.gpsimd.partition_broadcast(w_bc, wts)

        # delay regs: m = idx; jd = m>>7; u = 127 - m%128; delta = (24-jd)%24;
        # d = 128*delta + u.  One register set per engine, only for the taps
        # that engine actually combines.
        KPEC = int(os.environ.get("KPECOMB", "1"))
        if KPEC:
            # PE takes taps 1-4 as diag(w_i)-stationary matmuls (the adds
            # are free PSUM accumulation), DVE keeps taps 0,5,6,7.
            engines = {"v": mybir.EngineType.DVE,
                       "t": mybir.EngineType.PE}
            tap_sets = {"v": (0, 5, 6, 7), "t": (1, 2, 3, 4)}
        else:
            engines = {"v": mybir.EngineType.DVE,
                       "a": mybir.EngineType.Activation}
            tap_sets = {"v": (0, 3, 4, 5, 6, 7), "a": (1, 2)}
        delay_sv = {}
        for key, etype in engines.items():
            eng = nc.engines[etype]
            svs = {}
            for i in tap_sets[key]:
                regs = nc.alloc_registers(f"dly{key}{i}i{itag}", (etype,))
                nc.regs_load(regs, idx8[0:1, i:i + 1])
                r0 = regs.handles[0]
                t1 = eng.alloc_register(f"t1{key}_{i}_{itag}")
                t2 = eng.alloc_register(f"t2{key}_{i}_{itag}")
                eng.reg_alu(t1, r0, 128, ALU.divide)      # jd
                eng.reg_alu(t2, t1, 128, ALU.mult)
                eng.reg_alu(r0, r0, t2, ALU.subtract)     # m % 128
                eng.reg_alu(r0, 127, r0, ALU.subtract)    # u
                eng.reg_alu(t1, NT, t1, ALU.subtract)     # 24 - jd
                eng.reg_alu(t1, t1, NT, ALU.mod)          # delta
                eng.reg_alu(t1, t1, 128, ALU.mult)
                eng.reg_alu(t1, t1, r0, ALU.add)          # d
                svs[i] = nc.snap(t1, min_val=0, max_val=L - 1)
            delay_sv[key] = svs

        # diag(w_i) stationary tiles for the PE-side taps
        diag = {}
        if KPEC:
            ident = small_pool.tile([128, 128], F16, tag="ident",
                                    name=f"ident_{itag}")
            make_identity(nc, ident)
            for i in tap_sets["t"]:
                dg = small_pool.tile([128, 128], F16, tag=f"diag{i}",
                                     name=f"diag{i}_{itag}")
                nc.vector.tensor_scalar(dg, ident, w_bc[:, i:i + 1], None,
                                        op0=ALU.mult)
                diag[i] = dg

        # ---- remaining P-proj m-tiles + per-m combine (DVE/PE or DVE/ACT) --
        def combine_pe(m, p2t):
            """Taps 1-4 on PE: out_chunk = sum_i diag(w_i)^T @ pw[d_i+512c :
            .. +512] accumulated in PSUM (adds are free); DVE does taps
            0,5,6,7 and one merge add with the PE partial."""
            svs = delay_sv["v"]
            tsvs = delay_sv["t"]
            acc = acc_pool.tile([128, L], F16, tag="acc", name=f"acc_{m}_{itag}")
            t4 = acc_pool.tile([128, L], F16, tag="t4", name=f"t4_{m}_{itag}")
            pc = acc_pool.tile([128, L], F16, tag="pc", name=f"pc_{m}_{itag}")
            pw = p2t[:, 0:2 * L]
            for c in range(NC):
                # 3584-wide window so ds(sv, 512) stays in bounds for any
                # delay d_i in [0, L): only the offset is dynamic.
                pw_c = p2t[:, 512 * c: 512 * c + L + 512]
                pp = pc_pool.tile([128, 512], F32, tag="pcp",
                                  name=f"pcp_{m}_{c}_{itag}")
                for ii, i in enumerate(tap_sets["t"]):
                    nc.tensor.matmul(
                        pp,
                        diag[i],
                        pw_c[:, ds(tsvs[i], 512)],
                        start=(ii == 0), stop=(ii == len(tap_sets["t"]) - 1),
                    )
                nc.scalar.activation(pc[:, 512 * c: 512 * (c + 1)], pp,
                                     AF.Identity, bias=0.0, scale=1.0)
            nc.vector.tensor_scalar(acc, pw[:, ds(svs[0], L)],
                                    w_bc[:, 0:1], None, op0=ALU.mult)
            for i in (5, 6, 7):
                nc.vector.tensor_scalar(t4, pw[:, ds(svs[i], L)],
                                        w_bc[:, i:i + 1], None, op0=ALU.mult)
                nc.vector.tensor_add(acc, acc, t4)
            nc.vector.tensor_add(acc, acc, pc)
            nc.sync.dma_start(out[128 * m:128 * (m + 1), :], acc)

        def combine(m, p2t):
            if KPEC:
                return combine_pe(m, p2t)
            svs = delay_sv["v"]
            asvs = delay_sv["a"]
            acc = acc_pool.tile([128, L], F16, tag="acc", name=f"acc_{m}_{itag}")
            t_a = acc_pool.tile([128, L], F16, tag="t_a", name=f"ta_{m}_{itag}")
            t_b = acc_pool.tile([128, L], F16, tag="t_b", name=f"tb_{m}_{itag}")
            t4 = acc_pool.tile([128, L], F16, tag="t4", name=f"t4_{m}_{itag}")
            pw = p2t[:, 0:2 * L]
            # ACT: taps 1, 2 (activation scale)
            nc.scalar.activation(t_a, pw[:, ds(asvs[1], L)], AF.Identity,
                                 bias=0.0, scale=w_bc[:, 1:2])
            nc.scalar.activation(t_b, pw[:, ds(asvs[2], L)], AF.Identity,
                                 bias=0.0, scale=w_bc[:, 2:3])
            # DVE: all pointer-scaled taps (TensorScalarPtr is not legal on
            # Pool); taps 3,4 go to scratch tiles so the Pool engine can take
            # over 3 of the 7 merge adds (plain TensorTensor is Pool-legal).
            nc.vector.tensor_scalar(acc, pw[:, ds(svs[0], L)],
                                    w_bc[:, 0:1], None, op0=ALU.mult)
            for i in (5, 6, 7):
                nc.vector.tensor_scalar(t4, pw[:, ds(svs[i], L)],
                                        w_bc[:, i:i + 1], None, op0=ALU.mult)
                nc.vector.tensor_add(acc, acc, t4)
            if int(os.environ.get("KGPS", "0")):
                t3s = acc_pool.tile([128, L], F16, tag="t3s",
                                    name=f"t3s_{m}_{itag}")
                t4s = acc_pool.tile([128, L], F16, tag="t4s",
                                    name=f"t4s_{m}_{itag}")
                nc.vector.tensor_scalar(t3s, pw[:, ds(svs[3], L)],
                                        w_bc[:, 3:4], None, op0=ALU.mult)
                nc.vector.tensor_scalar(t4s, pw[:, ds(svs[4], L)],
                                        w_bc[:, 4:5], None, op0=ALU.mult)
                nc.gpsimd.tensor_add(t3s, t3s, t4s)
                nc.gpsimd.tensor_add(t3s, t3s, t_a)
                nc.gpsimd.tensor_add(t3s, t3s, t_b)
                nc.vector.tensor_add(acc, acc, t3s)
            else:
                for i in (3, 4):
                    nc.vector.tensor_scalar(t4, pw[:, ds(svs[i], L)],
                                            w_bc[:, i:i + 1], None,
                                            op0=ALU.mult)
                    nc.vector.tensor_add(acc, acc, t4)
                nc.vector.tensor_add(t_a, t_a, t_b)
                nc.vector.tensor_add(acc, acc, t_a)
            nc.sync.dma_start(out[128 * m:128 * (m + 1), :], acc)

        COMBM = int(os.environ.get("KCOMBM", "8"))
        for m in range(MT):
            p2t = p2ts[m] if m in p2ts else pproj(m)
            if m < COMBM:
                combine(m, p2t)

    est.close()


# ------------------------- host-side wrapper -------------------------
_CACHE = {}


def _build_runner(kiter=1, donate=True):
    """Build nc + a cached jitted SPMD callable (mirrors run_bass_via_pjrt).

    donate=False keeps the zero output buffers as ordinary (reusable) inputs:
    the kernel writes every element of `out`, so the pre-zeroed donation is
    only an XLA aliasing optimization, not a correctness requirement.  Timing
    harnesses use donate=False so staged device arrays can be reused across
    back-to-back dispatches."""
    import jax
    from jax.sharding import Mesh, PartitionSpec
    from jax.experimental.shard_map import shard_map
    from concourse import bass2jax
    import concourse.mybir as mb

    nc = build_nc(kiter=kiter)
    bass2jax.install_neuronx_cc_hook()

    partition_name = (nc.partition_id_tensor.name
                      if nc.partition_id_tensor else None)
    in_names, out_names, out_avals, zero_outs = [], [], [], []
    for alloc in nc.m.functions[0].allocations:
        if not isinstance(alloc, mb.MemoryLocationSet):
            continue
        name = alloc.memorylocations[0].name
        if alloc.kind == "ExternalInput":
            if name != partition_name:
                in_names.append(name)
        elif alloc.kind == "ExternalOutput":
            shape = tuple(alloc.tensor_shape)
            dtype = mb.dt.np(alloc.dtype)
            out_names.append(name)
            out_avals.append(jax.core.ShapedArray(shape, dtype))
            zero_outs.append(np.zeros(shape, dtype))
    n_params = len(in_names)
    all_names = list(in_names) + list(out_names)
    if partition_name is not None:
        all_names.append(partition_name)
    donate_nums = (tuple(range(n_params, n_params + len(out_names)))
                   if donate else ())

    def _body(*args):
        operands = list(args)
        if partition_name is not None:
            operands.append(bass2jax.partition_id_tensor())
        return tuple(bass2jax._bass_exec_p.bind(
            *operands,
            out_avals=tuple(out_avals),
            in_names=tuple(all_names),
            out_names=tuple(out_names),
            lowering_input_output_aliases=(),
            sim_require_finite=True,
            sim_require_nnan=True,
            nc=nc,
        ))

    devices = jax.devices()[:N_CORES]
    mesh = Mesh(np.asarray(devices), ("core",))
    in_specs = (PartitionSpec("core"),) * (n_params + len(out_names))
    out_specs = (PartitionSpec("core"),) * len(out_names)
    sharded = jax.jit(
        shard_map(_body, mesh=mesh, in_specs=in_specs, out_specs=out_specs,
                  check_rep=False),
        donate_argnums=donate_nums, keep_unused=True)
    return {
        "sharded": sharded, "in_names": in_names, "out_names": out_names,
        "out_avals": out_avals, "zero_outs": zero_outs,
    }


def _get_runner(kiter=1, donate=True):
    key = (kiter, donate)
    if key not in _CACHE:
        _CACHE[key] = _build_runner(kiter=kiter, donate=donate)
    return _CACHE[key]


def _concat_inputs(r, in_maps):
    per_core = [[np.asarray(m[name]) for name in r["in_names"]]
                for m in in_maps]
    concat_in = [np.concatenate([per_core[c][i] for c in range(N_CORES)],
                                axis=0)
                 for i in range(len(r["in_names"]))]
    concat_zeros = [np.zeros((N_CORES * z.shape[0], *z.shape[1:]), z.dtype)
                    for z in r["zero_outs"]]
    return concat_in, concat_zeros


def _run(r, concat_in, concat_zeros):
    out_arrs = r["sharded"](*concat_in, *concat_zeros)
    return [
        {name: np.asarray(out_arrs[i]).reshape(
            N_CORES, *r["out_avals"][i].shape)[c]
         for i, name in enumerate(r["out_names"])}
        for c in range(N_CORES)
    ]


def make_in_maps(queries, keys, values, Wq, bq, Wk, bk, Wv, bv, Wo, bo):
    """Pack full f32 inputs into per-core fp16 in_maps with folded weights.

    Wqk = Wq Wk^T: diag-sums of Q K^T equal diag-sums of Xq Wqk Xk^T up to a
    delay-independent constant (the bias cross-terms), which top-k ordering
    and softmax are both invariant to.  Wvo = Wv Wo (+ bvo = bv Wo + bo):
    the delay aggregation is a convex combination of time shifts and commutes
    with the channel projection, so V-proj and O-proj fuse into one matmul.
    """
    Wqk = (np.asarray(Wq, np.float64) @ np.asarray(Wk, np.float64).T)
    Wvo = (np.asarray(Wv, np.float64) @ np.asarray(Wo, np.float64))
    bvo = (np.asarray(bv, np.float64) @ np.asarray(Wo, np.float64)
           + np.asarray(bo, np.float64))
    wpack = np.concatenate([Wqk, Wvo], axis=0).astype(np.float16)
    bpack = bvo[None, :].astype(np.float32)
    queries = np.asarray(queries, np.float32)
    keys = np.asarray(keys, np.float32)
    values = np.asarray(values, np.float32)
    in_maps = []
    for b in range(B):
        xpack = np.concatenate(
            [queries[b], keys[b], values[b]], axis=0).astype(np.float16)
        in_maps.append({"xpack": xpack, "wpack": wpack, "bpack": bpack})
    return in_maps


def kernel(queries, keys, values, Wq, bq, Wk, bk, Wv, bv, Wo, bo):
    r = _get_runner(kiter=1)
    in_maps = make_in_maps(queries, keys, values, Wq, bq, Wk, bk, Wv, bv,
                           Wo, bo)
    concat_in, concat_zeros = _concat_inputs(r, in_maps)
    results = _run(r, concat_in, concat_zeros)
    outs = [results[b]["out"].T.astype(np.float32) for b in range(B)]
    return np.ascontiguousarray(np.stack(outs))


if __name__ == "__main__":
    rng = np.random.default_rng(0)
    ins = {
        "queries": rng.standard_normal((B, L, D)).astype(np.float32),
        "keys": rng.standard_normal((B, L, D)).astype(np.float32),
        "values": rng.standard_normal((B, L, D)).astype(np.float32),
        "Wq": (rng.standard_normal((D, D)) * 0.02).astype(np.float32),
        "bq": np.zeros(D, np.float32),
        "Wk": (rng.standard_normal((D, D)) * 0.02).astype(np.float32),
        "bk": np.zeros(D, np.float32),
        "Wv": (rng.standard_normal((D, D)) * 0.02).astype(np.float32),
        "bv": np.zeros(D, np.float32),
        "Wo": (rng.standard_normal((D, D)) * 0.02).astype(np.float32),
        "bo": np.zeros(D, np.float32),
    }
    o = kernel(**ins)
    print("out", o.shape, o.dtype, float(np.abs(o).max()))



# revision 30
# speedup vs baseline: 1.0644x; 1.0038x over previous
"""Trainium2 Bass kernel for the Autoformer autocorrelation block.

Contract: kernel(**inputs) takes FULL inputs (B=8 batches), returns FULL output
[8, 3072, 1024] f32. Internally: data-parallel over batch across 8 NeuronCores.

Weight folding (host side, fp64): the correlation only needs circular
diag-sums of Q K^T = Xq (Wq Wk^T) Xk^T + bias terms that are constant in the
delay, and top-k + softmax are invariant to constant shifts — so Wqk = Wq Wk^T
replaces the Q AND K projections with a single one.  The delay aggregation is
a convex combination of time shifts and commutes with the channel projection,
so Wvo = Wv Wo (bvo = bv Wo + bo) fuses the V and O projections.  PE work
drops from 5 to 2 projections + the Gram (22.5 -> 16.1 GMAC per core).

Per-core algorithm (one batch; t = time in [0,3072), d = channel in [0,1024)):
  1. XBAR DMA-transpose Xq (thirds, pipelined) and project with fp16 Wqk on
     the PE -> Q'^T in [d, t] layout; Xk^T and Xv^T are straight XBAR DMA
     transposes (no matmuls).  Issue order q -> k -> v matches consumer
     order since all transposes serialize on the shared XBAR path.
  2. diag-sums via Gram tiles on PE with block-diagonal ring accumulation
     (ring[jj] = sum of [128,128] blocks with (b-a)%24 == jj), then a
     strided-DMA "skew" through DRAM turns diagonals into columns and a
     PE ones-matmul reduce yields all 3072 diagonal sums at once (the dead
     ring tile's first row is reused as the colsum buffer).
  3. top-8 values+indices via DVE max/max_index; softmax on-device; delay
     values recovered with register ALU (DVE + ACT register sets).
  4. P^T = Wvo^T Xv^T + bvo (fp16 matmuls), written doubled along t from
     PSUM (ACT), per m-tile.
  5. out^T[d, t] = sum_i w_i * P^T[d, t + d_i] via runtime-register dynamic
     slices, split across engines: taps 1-4 on the PE as diag(w_i)-stationary
     matmuls whose adds are free PSUM accumulation (PE-register dynamic
     slices on the moving operand), taps 0,5-7 on DVE (4x tensor_scalar + 2x
     tensor_tensor), pipelined per m-tile against the P-projection.
     Host transposes back and upcasts fp16 -> f32.

Measured (loop-differenced, 8 cores): 702us baseline -> 572us.  Rejected by
measurement: Pool-engine merge adds in the combine (cross-engine semaphore
serialization, +90us), pre-emitting P-proj m-tiles before the colsum, and
DMA-based P^T doubling (both slightly negative).  fp8 Gram is numerically
unsafe: the rank-8/9 correlation gap is ~0.0025 sigma_c and fp8 input
quantization noise (~5% sigma_c) would flip top-k taps (output error ~0.5).

Timing support: build_nc(kiter=K) emits the body K times separated by
all-engine barriers, so test.py can measure the marginal per-iteration
hardware execution time ((t_K - t_1) / (K - 1)) with dispatch overhead
cancelled.
"""
import os
import sys

if "/opt/trn_rl_repo" not in sys.path:
    sys.path.insert(0, "/opt/trn_rl_repo")

import numpy as np

import concourse.bacc as bacc
import concourse.bass_utils as _bass_utils
import concourse.mybir as mybir
import concourse.tile as tile
from concourse.bass import ds
from concourse.bass_types import AP
from concourse.masks import make_identity

# KLDWOPT=1 re-enables walrus's redundant-LD_WEIGHTS elision
# ("--enable-ldw-opt=false" in this compile path) hoping to elide the
# Gram's repeated stationary loads under kt-major order — but walrus
# CRASHES in visitInstLdweights (CoreV3GenImpl.cpp:694) on this stream;
# the flag is hardcoded off for a reason.  Kept for documentation, off.
if not hasattr(_bass_utils, "_orig_run_command_ldw"):
    _bass_utils._orig_run_command_ldw = _bass_utils.run_command

    def _run_command_ldwopt(argv, *a, **kw):
        if int(os.environ.get("KLDWOPT", "0")) and isinstance(argv, list):
            argv = ["--enable-ldw-opt=true" if c == "--enable-ldw-opt=false"
                    else c for c in argv]
        return _bass_utils._orig_run_command_ldw(argv, *a, **kw)

    _bass_utils.run_command = _run_command_ldwopt

B, L, D = 8, 3072, 1024
NT = L // 128          # 24 t-blocks
NC = L // 512          # 6 t-chunks
KT = D // 128          # 8 contraction tiles
MT = D // 128          # 8 output-channel tiles
TOPK = 8
N_CORES = 8
WG = 3200              # ring width incl prepended block (25*128)
WS = WG + 127          # skew row width

F32 = mybir.dt.float32
F16 = mybir.dt.float16
U32 = mybir.dt.uint32
AF = mybir.ActivationFunctionType
ALU = mybir.AluOpType

# row offsets of q/k/v in xpack, and of Wqk/Wvo in wpack.
# Weight folding: diag-sums of Q K^T equal diag-sums of Xq (Wq Wk^T) Xk^T up
# to a delay-independent constant (bias terms), which softmax and top-k both
# ignore; and the output is sum_i w_i shift_{d_i}(Xv (Wv Wo)) + (bv Wo + bo)
# because the delay aggregation commutes with the channel projection and
# sum_i w_i = 1.  So only TWO projections remain: Q' = Xq Wqk and
# P = Xv Wvo + bvo.
XOFF = {"q": 0, "k": 1, "v": 2}
WOFF = {"qk": 0, "vo": 1}


def build_nc(kiter=1):
    nc = bacc.Bacc("TRN2", target_bir_lowering=False, debug=False,
                   num_devices=N_CORES)

    aps = {
        "xpack": nc.dram_tensor("xpack", [3 * L, D], F16,
                                kind="ExternalInput").ap(),
        "wpack": nc.dram_tensor("wpack", [2 * D, D], F16,
                                kind="ExternalInput").ap(),
        "bpack": nc.dram_tensor("bpack", [1, D], F32,
                                kind="ExternalInput").ap(),
    }
    out = nc.dram_tensor("out", [D, L], F16, kind="ExternalOutput").ap()
    skew = nc.dram_tensor("skew", [128 * WS + 256], F32)
    with tile.TileContext(nc) as tc:
        for it in range(kiter):
            _kernel_body(tc, nc, aps, out, skew, itag=str(it))
            if it < kiter - 1:
                tc.strict_bb_all_engine_barrier()
    nc.compile()
    return nc


def _load_weights16(nc, pool, w_dram, tag):
    """W [din, dout] fp16 -> SBUF fp16 [128, KT*D]; w16[p, kt*D+n] = W[kt*128+p, n]."""
    w16 = pool.tile([128, KT * D], F16, tag="w16", name=f"w16_{tag}")
    nc.sync.dma_start(w16.rearrange("p (a n) -> p a n", a=KT),
                      w_dram.rearrange("(a p) n -> p a n", p=128))
    return w16


def _transpose_chunk_dma(nc, x_dram, x_base, c, xtp):
    """XBAR DMA-transpose fp16 x rows [512c, 512(c+1)) straight from DRAM into
    xtp [128, KT*512] with xtp[p, kt*512 + j] = x[x_base + 512c + j, kt*128+p].

    KTRSPLIT=1 alternates issues between the SP and ACT HWDGE queues —
    measured INCORRECT output (ACT-issued transpose XBAR DMAs corrupt the
    result), so it stays off."""
    split = int(os.environ.get("KTRSPLIT", "0"))
    for kt in range(KT):
        eng = nc.scalar if (split and kt % 2) else nc.sync
        eng.dma_start_transpose(
            xtp[:, 512 * kt:512 * (kt + 1)],
            x_dram[x_base + 512 * c: x_base + 512 * (c + 1),
                   128 * kt:128 * (kt + 1)])


def _transpose_chunk(nc, ident, x_dram, x_base, c, xin_pool, tpsum_pool, xtp,
                     itag):
    """PE-transpose fp16 x rows [512c, 512(c+1)) into xtp [128, KT*512] with
    xtp[p, kt*512 + al*128 + j] = x[x_base + 512c + al*128 + j, kt*128 + p]."""
    for al in range(4):
        a = 4 * c + al
        x16 = xin_pool.tile([128, D], F16, tag="x16",
                            name=f"x16_{c}_{al}_{itag}")
        nc.sync.dma_start(x16, x_dram[x_base + 128 * a:x_base + 128 * (a + 1), :])
        for half in range(2):
            pt = tpsum_pool.tile([128, 512], F16, tag="tp",
                                 name=f"pt_{c}_{al}_{half}_{itag}")
            for k2 in range(4):
                dt = 4 * half + k2
                nc.tensor.transpose(
                    pt[:, 128 * k2:128 * (k2 + 1)],
                    x16[:, 128 * dt:128 * (dt + 1)],
                    ident,
                )
            dst = xtp.rearrange("p (k f) -> p k f", f=512)[
                :, 4 * half:4 * half + 4, 128 * al:128 * (al + 1)]
            src = pt.rearrange("p (k f) -> p k f", f=128)
            nc.vector.tensor_copy(dst, src)


def _load_bias(nc, pool, b_dram, tag):
    """bias [1, D] f32 -> SBUF [128, MT]; b_sb[p, m] = bias[m*128+p]."""
    b_sb = pool.tile([128, MT], F32, tag=tag, name=f"b_{tag}")
    nc.sync.dma_start(b_sb, b_dram.rearrange("o (m p) -> (o p) m", p=128))
    return b_sb


def _kernel_body(tc, nc, aps, out, skew, itag="0"):
    import contextlib
    PHASES = int(os.environ.get("KPHASES", "9"))
    est = contextlib.ExitStack()

    xpack, wpack, bpack = aps["xpack"], aps["wpack"], aps["bpack"]

    bias_pool = est.enter_context(tc.tile_pool(name=f"bias{itag}", bufs=1))
    small_pool = est.enter_context(tc.tile_pool(name=f"small{itag}", bufs=1))
    kv_pool = est.enter_context(tc.tile_pool(name=f"kv{itag}", bufs=1))
    ring_pool = est.enter_context(tc.tile_pool(name=f"ring{itag}", bufs=1))
    est_kt = contextlib.ExitStack()
    kt_pool = est_kt.enter_context(tc.tile_pool(name=f"ktp{itag}", bufs=1))
    qt_pool = est_kt.enter_context(tc.tile_pool(name=f"qtp{itag}", bufs=1))

    b_sb = {"o": _load_bias(nc, bias_pool, bpack[0:1, :], f"bo_{itag}")}

    kt_sb = kt_pool.tile([128, MT * L], F16, tag="kt",
                         name=f"kt_sb_{itag}")    # Xk^T, m-major
    qt_sb = qt_pool.tile([128, MT * L], F16, tag="qt",
                         name=f"qt_sb_{itag}")    # Q'^T = (Xq Wqk)^T, m-major
    vt_sb = kv_pool.tile([128, MT * L], F16, tag="vt",
                         name=f"vt_sb_{itag}")    # Xv^T, m-major

    # ------- Phase 1: Xq^T transpose + Q' projection; Xk^T/Xv^T transpose ---
    TH = L // 3
    with tc.tile_pool(name=f"wpool{itag}", bufs=1) as wpool, \
         tc.tile_pool(name=f"xtp{itag}", bufs=2) as xtp_pool, \
         tc.tile_pool(name=f"ppsum{itag}", bufs=4, space="PSUM") as ppsum_pool:
        w16 = _load_weights16(nc, wpool,
                              wpack[WOFF["qk"] * D:(WOFF["qk"] + 1) * D, :],
                              f"wqk_{itag}")
        # Xq first (its transposes gate the only phase-1 PE work), then Xk
        # (gates the Gram), then Xv (only needed by the P-projection, which
        # runs after the Gram).  All XBAR transposes serialize on the shared
        # DMA/XBAR path (~43us per signal), so issue order = consumer order.
        x_base = XOFF["q"] * L
        for h in range(3):
            xtp = xtp_pool.tile([128, KT * TH], F16, tag="xtp3",
                                name=f"xtp3_q_{h}_{itag}")
            for kt in range(KT):
                nc.sync.dma_start_transpose(
                    xtp[:, TH * kt:TH * (kt + 1)],
                    xpack[x_base + TH * h: x_base + TH * (h + 1),
                          128 * kt:128 * (kt + 1)])
            for cc in range(2):
                c = 2 * h + cc
                for m in range(MT):
                    pp = ppsum_pool.tile([128, 512], F32, tag="pp",
                                         name=f"pp_q_{c}_{m}_{itag}")
                    for kt in range(KT):
                        nc.tensor.matmul(
                            pp,
                            w16[:, kt * D + 128 * m:
                                kt * D + 128 * (m + 1)],
                            xtp[:, TH * kt + 512 * cc:
                                TH * kt + 512 * (cc + 1)],
                            start=(kt == 0), stop=(kt == KT - 1),
                        )
                    nc.scalar.activation(
                        qt_sb[:, m * L + 512 * c: m * L + 512 * (c + 1)],
                        pp, AF.Identity, bias=0.0, scale=1.0)
        # Xk^T / Xv^T: straight XBAR DMA transposes into kt_sb / vt_sb
        # (no projection matmuls — the Gram consumes raw Xk^T and the
        # P-projection consumes raw Xv^T with the folded Wvo).
        for which, dst in (("k", kt_sb), ("v", vt_sb)):
            x_base = XOFF[which] * L
            for h in range(3):
                for kt in range(KT):
                    nc.sync.dma_start_transpose(
                        dst[:, kt * L + TH * h: kt * L + TH * (h + 1)],
                        xpack[x_base + TH * h: x_base + TH * (h + 1),
                              128 * kt:128 * (kt + 1)])

    if PHASES < 2:
        est_kt.close(); est.close(); return

    # ---------------- Phase 2: Gram + block-diagonal ring ----------------
    ring = ring_pool.tile([128, WG], F32, tag="ring", name=f"ring_{itag}")
    # KPSRING=1 accumulates the ring directly in PSUM (no DVE drains, exact
    # same math — needs the rotated a-loop because the matmul start bit
    # zeroes the whole target region).  Measured 606us vs 572us for the
    # default path: the +36% matmul instruction count (wrap splits) costs
    # more than the removed DVE drains, i.e. the drains were already hidden
    # and the Gram's overhead is per-matmul weight loads.  Kept off.
    if int(os.environ.get("KPSRING", "0")):
        # Accumulate the ring DIRECTLY in PSUM across a-tiles: ring column
        # rc (slot jj = rc//128 - 1, lane u = rc%128) needs
        #   sum_a sum_d qt[d, 128a+p] * kt[d, (128(a-1) + rc) % L],
        # i.e. for each (chunk, a, kt) one moving slice of Xk^T that is
        # contiguous except at the mod-L wrap (<= 2 matmul pieces).  This
        # removes all 144 DVE ring drains and their PSUM-reuse stalls; every
        # column's first/last contribution is at a=0 / a=NT-1, so start/stop
        # flags are uniform per a.
        with tc.tile_pool(name=f"gpsum{itag}", bufs=2,
                          space="PSUM") as gpsum_pool:
            for h in range((WG + 511) // 512):
                rc0 = 512 * h
                w = min(512, WG - rc0)
                rg = gpsum_pool.tile([128, w], F32, tag="rg",
                                     name=f"rg{h}_{itag}")
                # rotate the a-loop so the FIRST step is single-piece: the
                # start bit zeroes the whole target region, so a two-piece
                # first step would wipe its own first piece.
                a_start = next(a for a in range(NT)
                               if (128 * (a - 1) + rc0) % L + w <= L)
                for ai in range(NT):
                    a = (a_start + ai) % NT
                    for kt in range(KT):
                        t0 = (128 * (a - 1) + rc0) % L
                        if t0 + w <= L:
                            pieces = ((0, t0, w),)
                        else:
                            w1 = L - t0
                            pieces = ((0, t0, w1), (w1, 0, w - w1))
                        for po, ts, pwid in pieces:
                            nc.tensor.matmul(
                                rg[:, po:po + pwid],
                                qt_sb[:, kt * L + 128 * a:
                                      kt * L + 128 * (a + 1)],
                                kt_sb[:, kt * L + ts: kt * L + ts + pwid],
                                start=(ai == 0 and kt == 0),
                                stop=(ai == NT - 1 and kt == KT - 1),
                                skip_group_check=True,
                            )
                nc.vector.tensor_copy(ring[:, rc0:rc0 + w], rg)
        est_kt.close()  # K^T / Q'^T no longer needed
        if PHASES < 3:
            est.close(); return
        return _phase345(tc, nc, est, out, skew, ring, vt_sb, b_sb,
                         small_pool, wpack, itag)

    nc.vector.memset(ring, 0.0)
    with tc.tile_pool(name=f"gpsum{itag}", bufs=1, space="PSUM") as gpsum_pool:
        for a in range(NT):
            gps = [gpsum_pool.tile([128, 512], F32, tag=f"gp{c}",
                                   name=f"gp{a}_{c}_{itag}")
                   for c in range(NC)]
            if int(os.environ.get("KCMAJ", "1")):
                # c-major: each psum tile finishes early so its ring add
                # (DVE) overlaps the next tile's matmuls instead of
                # stalling the a+1 accumulation group on psum reuse.
                for c in range(NC):
                    for kt in range(KT):
                        nc.tensor.matmul(
                            gps[c],
                            qt_sb[:, kt * L + 128 * a:
                                  kt * L + 128 * (a + 1)],
                            kt_sb[:, kt * L + 512 * c:
                                  kt * L + 512 * (c + 1)],
                            start=(kt == 0), stop=(kt == KT - 1),
                        )
            else:
                for kt in range(KT):
                    for c in range(NC):
                        nc.tensor.matmul(
                            gps[c],
                            qt_sb[:, kt * L + 128 * a:
                                  kt * L + 128 * (a + 1)],
                            kt_sb[:, kt * L + 512 * c:
                                  kt * L + 512 * (c + 1)],
                            start=(kt == 0), stop=(kt == KT - 1),
                        )
            if int(os.environ.get("KNORING", "0")):
                # timing-only diagnostic: drop the ring drains entirely to
                # measure the pure Gram matmul stream (output is garbage)
                continue
            RD = int(os.environ.get("KRDRAIN", "0"))

            def radd(dst, in1, gp_slice, which):
                # KRDRAIN: split every drain column-wise between DVE and
                # Pool.  The two engines touch disjoint column ranges, so
                # each engine's RAW chain on `ring` stays engine-local (no
                # cross-engine semaphores on the accumulation path).
                if not RD:
                    nc.vector.tensor_add(dst, in1, gp_slice)
                    return
                n = dst.shape[-1]
                h = (n // 2) // 128 * 128 or n
                nc.vector.tensor_add(dst[:, :h], in1[:, :h], gp_slice[:, :h])
                if h < n:
                    nc.gpsimd.tensor_add(dst[:, h:], in1[:, h:],
                                         gp_slice[:, h:])

            for c in range(NC):
                gp = gps[c]
                jj0 = (4 * c - a) % NT
                off = 128 * (jj0 + 1)
                if jj0 <= NT - 4:
                    radd(ring[:, off:off + 512], ring[:, off:off + 512], gp,
                         c)
                else:
                    w1 = 128 * (NT - jj0)
                    radd(ring[:, off:off + w1], ring[:, off:off + w1],
                         gp[:, :w1], c)
                    radd(ring[:, 128:128 + 512 - w1],
                         ring[:, 128:128 + 512 - w1], gp[:, w1:], c)
    # ring block jj lives at offset 128*(jj+1); prepend a copy of block 23
    nc.vector.tensor_copy(ring[:, 0:128], ring[:, 128 * NT:128 * (NT + 1)])
    est_kt.close()  # K^T / Q'^T no longer needed
    if PHASES < 3:
        est.close(); return
    return _phase345(tc, nc, est, out, skew, ring, vt_sb, b_sb, small_pool,
                     wpack, itag)


def _phase345(tc, nc, est, out, skew, ring, vt_sb, b_sb, small_pool, wpack,
              itag):
    # ------ Phases 3-5: P-projection overlapped with skew -> top-8 ---------
    # The skew DMA round trip + colsum + max/softmax/register chain is a
    # serial ~25us tail that would idle the PE: emit the first KPRE m-tiles
    # of the P-projection BEFORE the colsum so the PE stays busy through it.
    PHASES = int(os.environ.get("KPHASES", "9"))
    PRE = max(0, min(int(os.environ.get("KPRE", "0")), MT))
    KDBL = int(os.environ.get("KDBLACT", "1"))
    with tc.tile_pool(name=f"wos{itag}", bufs=1) as wos_pool, \
         tc.tile_pool(name=f"p2tp{itag}", bufs=3) as p2t_pool, \
         tc.tile_pool(name=f"ppsum4{itag}", bufs=2, space="PSUM") as ppsum_pool, \
         tc.tile_pool(name=f"pcpsum{itag}", bufs=2, space="PSUM") as pc_pool, \
         tc.tile_pool(name=f"accp{itag}", bufs=2) as acc_pool, \
         tc.tile_pool(name=f"skp{itag}", bufs=1) as sk_pool, \
         tc.tile_pool(name=f"cspsum{itag}", bufs=1, space="PSUM") as cs_pool:
        wo16 = _load_weights16(nc, wos_pool,
                               wpack[WOFF["vo"] * D:(WOFF["vo"] + 1) * D, :],
                               f"wo_{itag}")

        def pproj(m):
            """P^T m-tile: matmuls + bias, doubled along t for the runtime
            circular slice (double via DMA on the otherwise idle SP queue
            unless KDBLACT=1 re-enables the ACT double-write)."""
            p2t = p2t_pool.tile([128, 2 * L], F16, tag="p2t",
                                name=f"p2t_{m}_{itag}")
            for c in range(NC):
                pp = ppsum_pool.tile([128, 512], F32, tag="pp",
                                     name=f"pp4_{c}_{m}_{itag}")
                for kt in range(KT):
                    nc.tensor.matmul(
                        pp,
                        wo16[:, kt * D + 128 * m: kt * D + 128 * (m + 1)],
                        vt_sb[:, kt * L + 512 * c: kt * L + 512 * (c + 1)],
                        start=(kt == 0), stop=(kt == KT - 1),
                    )
                nc.scalar.activation(
                    p2t[:, 512 * c: 512 * (c + 1)],
                    pp, AF.Identity, bias=b_sb["o"][:, m:m + 1], scale=1.0)
                if KDBL:
                    nc.scalar.activation(
                        p2t[:, L + 512 * c: L + 512 * (c + 1)],
                        pp, AF.Identity, bias=b_sb["o"][:, m:m + 1],
                        scale=1.0)
            if not KDBL:
                nc.sync.dma_start(p2t[:, L:2 * L], p2t[:, 0:L])
            return p2t

        p2ts = {}
        for m in range(PRE):
            p2ts[m] = pproj(m)

        # ---- skew -> colsum -> top-8 (DMA/PE-colsum/DVE under P-proj) ----
        sk_sb = sk_pool.tile([128, L], F32, tag="sk", name=f"sk_{itag}")
        skew_rd = AP(tensor=skew, offset=128, ap=[[WS, 128], [1, L]])
        skew_wr = AP(tensor=skew, offset=127, ap=[[WS - 1, 128], [1, WG]])
        nc.sync.dma_start(skew_wr, ring[:, 0:WG])    # skewed write
        nc.sync.dma_start(sk_sb, skew_rd)            # read back
        ones = sk_pool.tile([128, 1], F32, tag="ones", name=f"ones_{itag}")
        nc.vector.memset(ones, 1.0)
        # ring is dead once the skew write has read it — reuse its first row
        # as the colsum buffer (Tile orders the WAR hazard on the slice).
        colsum = ring[0:1, 0:L]
        for half in range(2):
            cs_psum = cs_pool.tile([1, L // 2], F32, tag="cs",
                                   name=f"cs_{half}_{itag}")
            for ch in range(NC // 2):
                nc.tensor.matmul(
                    cs_psum[:, 512 * ch:512 * (ch + 1)],
                    ones,
                    sk_sb[:, half * (L // 2) + 512 * ch:
                          half * (L // 2) + 512 * (ch + 1)],
                    start=True, stop=True,
                )
            nc.vector.tensor_copy(colsum[:, half * (L // 2):
                                         (half + 1) * (L // 2)], cs_psum)
        max8 = small_pool.tile([1, TOPK], F32, tag="max8", name=f"max8_{itag}")
        idx8 = small_pool.tile([1, TOPK], U32, tag="idx8", name=f"idx8_{itag}")
        sl = colsum[0:1, 0:L]
        nc.vector.max(out=max8, in_=sl)
        nc.vector.max_index(idx8, max8, sl)
        if PHASES < 4:
            est.close(); return

        # softmax(max8 / D)
        wts = small_pool.tile([1, TOPK], F32, tag="wts", name=f"wts_{itag}")
        negmax = small_pool.tile([1, 1], F32, tag="negmax",
                                 name=f"negmax_{itag}")
        inv = small_pool.tile([1, 1], F32, tag="inv", name=f"inv_{itag}")
        nc.vector.tensor_scalar_mul(negmax, max8[0:1, 0:1], -1.0 / D)
        nc.scalar.activation(wts, max8, AF.Exp, bias=negmax[0:1, 0:1],
                             scale=1.0 / D)
        nc.vector.reduce_sum(inv, wts, axis=mybir.AxisListType.X)
        nc.vector.reciprocal(inv, inv)
        nc.vector.tensor_scalar(wts, wts, inv[0:1, 0:1], None, op0=ALU.mult)
        w_bc = small_pool.tile([128, TOPK], F32, tag="wbc", name=f"wbc_{itag}")
        nc.gpsimd.partition_broadcast(w_bc, wts)

        # delay regs: m = idx; jd = m>>7; u = 127 - m%128; delta = (24-jd)%24;
        # d = 128*delta + u.  One register set per engine, only for the taps
        # that engine actually combines.
        KPEC = int(os.environ.get("KPECOMB", "1"))
        if KPEC:
            # PE takes taps 1-4 as diag(w_i)-stationary matmuls (the adds
            # are free PSUM accumulation), DVE keeps taps 0,5,6,7.
            engines = {"v": mybir.EngineType.DVE,
                       "t": mybir.EngineType.PE}
            tap_sets = {"v": (0, 5, 6, 7), "t": (1, 2, 3, 4)}
        else:
            engines = {"v": mybir.EngineType.DVE,
                       "a": mybir.EngineType.Activation}
            tap_sets = {"v": (0, 3, 4, 5, 6, 7), "a": (1, 2)}
        delay_sv = {}
        for key, etype in engines.items():
            eng = nc.engines[etype]
            svs = {}
            for i in tap_sets[key]:
                regs = nc.alloc_registers(f"dly{key}{i}i{itag}", (etype,))
                nc.regs_load(regs, idx8[0:1, i:i + 1])
                r0 = regs.handles[0]
                t1 = eng.alloc_register(f"t1{key}_{i}_{itag}")
                t2 = eng.alloc_register(f"t2{key}_{i}_{itag}")
                eng.reg_alu(t1, r0, 128, ALU.divide)      # jd
                eng.reg_alu(t2, t1, 128, ALU.mult)
                eng.reg_alu(r0, r0, t2, ALU.subtract)     # m % 128
                eng.reg_alu(r0, 127, r0, ALU.subtract)    # u
                eng.reg_alu(t1, NT, t1, ALU.subtract)     # 24 - jd
                eng.reg_alu(t1, t1, NT, ALU.mod)          # delta
                eng.reg_alu(t1, t1, 128, ALU.mult)
                eng.reg_alu(t1, t1, r0, ALU.add)          # d
                svs[i] = nc.snap(t1, min_val=0, max_val=L - 1)
            delay_sv[key] = svs

        # diag(w_i) stationary tiles for the PE-side taps
        diag = {}
        if KPEC:
            ident = small_pool.tile([128, 128], F16, tag="ident",
                                    name=f"ident_{itag}")
            make_identity(nc, ident)
            for i in tap_sets["t"]:
                dg = small_pool.tile([128, 128], F16, tag=f"diag{i}",
                                     name=f"diag{i}_{itag}")
                nc.vector.tensor_scalar(dg, ident, w_bc[:, i:i + 1], None,
                                        op0=ALU.mult)
                diag[i] = dg

        # ---- remaining P-proj m-tiles + per-m combine (DVE/PE or DVE/ACT) --
        def combine_pe(m, p2t):
            """Taps 1-4 on PE: out_chunk = sum_i diag(w_i)^T @ pw[d_i+512c :
            .. +512] accumulated in PSUM (adds are free); DVE does taps
            0,5,6,7 and one merge add with the PE partial."""
            svs = delay_sv["v"]
            tsvs = delay_sv["t"]
            acc = acc_pool.tile([128, L], F16, tag="acc", name=f"acc_{m}_{itag}")
            t4 = acc_pool.tile([128, L], F16, tag="t4", name=f"t4_{m}_{itag}")
            pc = acc_pool.tile([128, L], F16, tag="pc", name=f"pc_{m}_{itag}")
            pw = p2t[:, 0:2 * L]
            for c in range(NC):
                # 3584-wide window so ds(sv, 512) stays in bounds for any
                # delay d_i in [0, L): only the offset is dynamic.
                pw_c = p2t[:, 512 * c: 512 * c + L + 512]
                pp = pc_pool.tile([128, 512], F32, tag="pcp",
                                  name=f"pcp_{m}_{c}_{itag}")
                for ii, i in enumerate(tap_sets["t"]):
                    nc.tensor.matmul(
                        pp,
                        diag[i],
                        pw_c[:, ds(tsvs[i], 512)],
                        start=(ii == 0), stop=(ii == len(tap_sets["t"]) - 1),
                    )
                nc.scalar.activation(pc[:, 512 * c: 512 * (c + 1)], pp,
                                     AF.Identity, bias=0.0, scale=1.0)
            nc.vector.tensor_scalar(acc, pw[:, ds(svs[0], L)],
                                    w_bc[:, 0:1], None, op0=ALU.mult)
            for i in (5, 6, 7):
                nc.vector.tensor_scalar(t4, pw[:, ds(svs[i], L)],
                                        w_bc[:, i:i + 1], None, op0=ALU.mult)
                nc.vector.tensor_add(acc, acc, t4)
            nc.vector.tensor_add(acc, acc, pc)
            nc.sync.dma_start(out[128 * m:128 * (m + 1), :], acc)

        def combine(m, p2t):
            if KPEC:
                return combine_pe(m, p2t)
            svs = delay_sv["v"]
            asvs = delay_sv["a"]
            acc = acc_pool.tile([128, L], F16, tag="acc", name=f"acc_{m}_{itag}")
            t_a = acc_pool.tile([128, L], F16, tag="t_a", name=f"ta_{m}_{itag}")
            t_b = acc_pool.tile([128, L], F16, tag="t_b", name=f"tb_{m}_{itag}")
            t4 = acc_pool.tile([128, L], F16, tag="t4", name=f"t4_{m}_{itag}")
            pw = p2t[:, 0:2 * L]
            # ACT: taps 1, 2 (activation scale)
            nc.scalar.activation(t_a, pw[:, ds(asvs[1], L)], AF.Identity,
                                 bias=0.0, scale=w_bc[:, 1:2])
            nc.scalar.activation(t_b, pw[:, ds(asvs[2], L)], AF.Identity,
                                 bias=0.0, scale=w_bc[:, 2:3])
            # DVE: all pointer-scaled taps (TensorScalarPtr is not legal on
            # Pool); taps 3,4 go to scratch tiles so the Pool engine can take
            # over 3 of the 7 merge adds (plain TensorTensor is Pool-legal).
            nc.vector.tensor_scalar(acc, pw[:, ds(svs[0], L)],
                                    w_bc[:, 0:1], None, op0=ALU.mult)
            for i in (5, 6, 7):
                nc.vector.tensor_scalar(t4, pw[:, ds(svs[i], L)],
                                        w_bc[:, i:i + 1], None, op0=ALU.mult)
                nc.vector.tensor_add(acc, acc, t4)
            if int(os.environ.get("KGPS", "0")):
                t3s = acc_pool.tile([128, L], F16, tag="t3s",
                                    name=f"t3s_{m}_{itag}")
                t4s = acc_pool.tile([128, L], F16, tag="t4s",
                                    name=f"t4s_{m}_{itag}")
                nc.vector.tensor_scalar(t3s, pw[:, ds(svs[3], L)],
                                        w_bc[:, 3:4], None, op0=ALU.mult)
                nc.vector.tensor_scalar(t4s, pw[:, ds(svs[4], L)],
                                        w_bc[:, 4:5], None, op0=ALU.mult)
                nc.gpsimd.tensor_add(t3s, t3s, t4s)
                nc.gpsimd.tensor_add(t3s, t3s, t_a)
                nc.gpsimd.tensor_add(t3s, t3s, t_b)
                nc.vector.tensor_add(acc, acc, t3s)
            else:
                for i in (3, 4):
                    nc.vector.tensor_scalar(t4, pw[:, ds(svs[i], L)],
                                            w_bc[:, i:i + 1], None,
                                            op0=ALU.mult)
                    nc.vector.tensor_add(acc, acc, t4)
                nc.vector.tensor_add(t_a, t_a, t_b)
                nc.vector.tensor_add(acc, acc, t_a)
            nc.sync.dma_start(out[128 * m:128 * (m + 1), :], acc)

        COMBM = int(os.environ.get("KCOMBM", "8"))
        for m in range(MT):
            p2t = p2ts[m] if m in p2ts else pproj(m)
            if m < COMBM:
                combine(m, p2t)

    est.close()


# ------------------------- host-side wrapper -------------------------
_CACHE = {}


def _build_runner(kiter=1, donate=True):
    """Build nc + a cached jitted SPMD callable (mirrors run_bass_via_pjrt).

    donate=False keeps the zero output buffers as ordinary (reusable) inputs:
    the kernel writes every element of `out`, so the pre-zeroed donation is
    only an XLA aliasing optimization, not a correctness requirement.  Timing
    harnesses use donate=False so staged device arrays can be reused across
    back-to-back dispatches."""
    import jax
    from jax.sharding import Mesh, PartitionSpec
    from jax.experimental.shard_map import shard_map
    from concourse import bass2jax
    import concourse.mybir as mb

    nc = build_nc(kiter=kiter)
    bass2jax.install_neuronx_cc_hook()

    partition_name = (nc.partition_id_tensor.name
                      if nc.partition_id_tensor else None)
    in_names, out_names, out_avals, zero_outs = [], [], [], []
    for alloc in nc.m.functions[0].allocations:
        if not isinstance(alloc, mb.MemoryLocationSet):
            continue
        name = alloc.memorylocations[0].name
        if alloc.kind == "ExternalInput":
            if name != partition_name:
                in_names.append(name)
        elif alloc.kind == "ExternalOutput":
            shape = tuple(alloc.tensor_shape)
            dtype = mb.dt.np(alloc.dtype)
            out_names.append(name)
            out_avals.append(jax.core.ShapedArray(shape, dtype))
            zero_outs.append(np.zeros(shape, dtype))
    n_params = len(in_names)
    all_names = list(in_names) + list(out_names)
    if partition_name is not None:
        all_names.append(partition_name)
    donate_nums = (tuple(range(n_params, n_params + len(out_names)))
                   if donate else ())

    def _body(*args):
        operands = list(args)
        if partition_name is not None:
            operands.append(bass2jax.partition_id_tensor())
        return tuple(bass2jax._bass_exec_p.bind(
            *operands,
            out_avals=tuple(out_avals),
            in_names=tuple(all_names),
            out_names=tuple(out_names),
            lowering_input_output_aliases=(),
            sim_require_finite=True,
            sim_require_nnan=True,
            nc=nc,
        ))

    devices = jax.devices()[:N_CORES]
    mesh = Mesh(np.asarray(devices), ("core",))
    in_specs = (PartitionSpec("core"),) * (n_params + len(out_names))
    out_specs = (PartitionSpec("core"),) * len(out_names)
    sharded = jax.jit(
        shard_map(_body, mesh=mesh, in_specs=in_specs, out_specs=out_specs,
                  check_rep=False),
        donate_argnums=donate_nums, keep_unused=True)
    return {
        "sharded": sharded, "in_names": in_names, "out_names": out_names,
        "out_avals": out_avals, "zero_outs": zero_outs,
    }


def _get_runner(kiter=1, donate=True):
    key = (kiter, donate)
    if key not in _CACHE:
        _CACHE[key] = _build_runner(kiter=kiter, donate=donate)
    return _CACHE[key]


def _concat_inputs(r, in_maps):
    per_core = [[np.asarray(m[name]) for name in r["in_names"]]
                for m in in_maps]
    concat_in = [np.concatenate([per_core[c][i] for c in range(N_CORES)],
                                axis=0)
                 for i in range(len(r["in_names"]))]
    concat_zeros = [np.zeros((N_CORES * z.shape[0], *z.shape[1:]), z.dtype)
                    for z in r["zero_outs"]]
    return concat_in, concat_zeros


def _run(r, concat_in, concat_zeros):
    out_arrs = r["sharded"](*concat_in, *concat_zeros)
    return [
        {name: np.asarray(out_arrs[i]).reshape(
            N_CORES, *r["out_avals"][i].shape)[c]
         for i, name in enumerate(r["out_names"])}
        for c in range(N_CORES)
    ]


def make_in_maps(queries, keys, values, Wq, bq, Wk, bk, Wv, bv, Wo, bo):
    """Pack full f32 inputs into per-core fp16 in_maps with folded weights.

    Wqk = Wq Wk^T: diag-sums of Q K^T equal diag-sums of Xq Wqk Xk^T up to a
    delay-independent constant (the bias cross-terms), which top-k ordering
    and softmax are both invariant to.  Wvo = Wv Wo (+ bvo = bv Wo + bo):
    the delay aggregation is a convex combination of time shifts and commutes
    with the channel projection, so V-proj and O-proj fuse into one matmul.
    """
    Wqk = (np.asarray(Wq, np.float64) @ np.asarray(Wk, np.float64).T)
    Wvo = (np.asarray(Wv, np.float64) @ np.asarray(Wo, np.float64))
    bvo = (np.asarray(bv, np.float64) @ np.asarray(Wo, np.float64)
           + np.asarray(bo, np.float64))
    wpack = np.concatenate([Wqk, Wvo], axis=0).astype(np.float16)
    bpack = bvo[None, :].astype(np.float32)
    queries = np.asarray(queries, np.float32)
    keys = np.asarray(keys, np.float32)
    values = np.asarray(values, np.float32)
    in_maps = []
    for b in range(B):
        xpack = np.concatenate(
            [queries[b], keys[b], values[b]], axis=0).astype(np.float16)
        in_maps.append({"xpack": xpack, "wpack": wpack, "bpack": bpack})
    return in_maps


def kernel(queries, keys, values, Wq, bq, Wk, bk, Wv, bv, Wo, bo):
    r = _get_runner(kiter=1)
    in_maps = make_in_maps(queries, keys, values, Wq, bq, Wk, bk, Wv, bv,
                           Wo, bo)
    concat_in, concat_zeros = _concat_inputs(r, in_maps)
    results = _run(r, concat_in, concat_zeros)
    outs = [results[b]["out"].T.astype(np.float32) for b in range(B)]
    return np.ascontiguousarray(np.stack(outs))


if __name__ == "__main__":
    rng = np.random.default_rng(0)
    ins = {
        "queries": rng.standard_normal((B, L, D)).astype(np.float32),
        "keys": rng.standard_normal((B, L, D)).astype(np.float32),
        "values": rng.standard_normal((B, L, D)).astype(np.float32),
        "Wq": (rng.standard_normal((D, D)) * 0.02).astype(np.float32),
        "bq": np.zeros(D, np.float32),
        "Wk": (rng.standard_normal((D, D)) * 0.02).astype(np.float32),
        "bk": np.zeros(D, np.float32),
        "Wv": (rng.standard_normal((D, D)) * 0.02).astype(np.float32),
        "bv": np.zeros(D, np.float32),
        "Wo": (rng.standard_normal((D, D)) * 0.02).astype(np.float32),
        "bo": np.zeros(D, np.float32),
    }
    o = kernel(**ins)
    print("out", o.shape, o.dtype, float(np.abs(o).max()))

